# revision 29
# baseline (speedup 1.0000x reference)
"""Trainium2 Bass kernel for MultiHeadSelfAttention with RoPE.

Problem: x[2, 2048, 1024] @ W_qkv[1024, 3072] -> rope(q,k) -> softmax(q k^T/8) v
         -> out @ W_out[1024, 1024].

Sharding (8 cores): batch (2-way) x head-group (4-way, 4 heads each).
Each core computes a partial output [2048, 1024] = attnout_heads @ W_out_rows;
host sums the 4 head-group partials per batch.

All matmul operands use float32r (TF32-like fp32: full-rate on the PE vs 4x
slower for plain fp32, ~1.5e-4 relative error). PSUM accumulation is fp32.

On-core dataflow is fully "transposed" so the PE never needs a transpose:
  qT,kT[c, s] = sum_e W[e, c] * xT[e, s]   (lhsT = W slice, rhs = xT)
  rot = Mswap @ qT (PE), q' = qT*cos + rot*sin_signed (DVE)
  scoresT[sk, sq] = sum_d kT[d, sk] qT[d, sq]  (2 heads row-packed, K=64)
  attnT = exp(scoresT/8) (ScalarE, PSUM->SBUF)
  outT[d, sq] += sum_sk v[sk, d] attnT[sk, sq] (2 heads col-packed, PSUM accum)
  denom[sq]  += sum_sk attnT[sk, sq]           (ones-column matmuls, packed)
  attnout = outT * (1/denom)  -> out_partial[s, e] = attnoutT.T @ W_out_rows
"""

import sys

if "/opt/trn_rl_repo" not in sys.path:
    sys.path.insert(0, "/opt/trn_rl_repo")

import numpy as np

B, S, E = 2, 2048, 1024
ATT = 1024
H = 16
D = 64
HG = 4            # head groups (cores per batch)
HPG = H // HG     # heads per core = 4
PAIRS = HPG // 2  # head pairs per core = 2
ROPE_THETA = 10000.0
N_CORES = 8

SQ_CHUNK = 1024   # sq chunk for exp / attn@v psum tiles
NQ = SQ_CHUNK // 512  # matmuls of N=512 per chunk
N_SK = S // 128   # 16 sk tiles
N_CH = S // SQ_CHUNK  # 2 chunks

# Schraudolph fast-exp: i32 = int(A*s + B); bitcast(i32) ~ exp(0.125*s) with
# +-1.8% rms sawtooth error. C = 482804 was calibrated on this hardware
# (zero mean log error, so fast-exp'd softmax weights are unbiased vs the
# ScalarE ACT-exp'd ones; numerator and denominator use the same values so
# softmax normalization is consistent). Half the exps move off the ScalarE
# critical path onto the otherwise-idle DVE (int math, also releases the
# scores-PSUM WAR early) + gpsimd (bitcast -> bf16 convert).
SCH_A = 0.125 * 12102203.161561485   # 0.125 * 2^23/ln2
SCH_B = 1065353216.0 - 482804.0      # 127*2^23 - C

_BUILT = {}


def _build_program(dbg=False):
    import concourse.bacc as bacc
    import concourse.tile as tile
    import concourse.mybir as mybir

    f32 = mybir.dt.float32
    f32r = mybir.dt.float32r
    bf16 = mybir.dt.bfloat16
    i32 = mybir.dt.int32
    AF = mybir.ActivationFunctionType
    ALU = mybir.AluOpType

    nc = bacc.Bacc(
        "TRN2",
        target_bir_lowering=False,
        debug=False,
        enable_asserts=False,
        num_devices=N_CORES,
    )

    xT = nc.dram_tensor("xT", [E, S], bf16, kind="ExternalInput").ap()
    w_qk = nc.dram_tensor("w_qk", [E, 2 * HPG * D], bf16, kind="ExternalInput").ap()
    w_v = nc.dram_tensor("w_v", [E, HPG * D], bf16, kind="ExternalInput").ap()
    w_o = nc.dram_tensor("w_o", [HPG * D, E], f32r, kind="ExternalInput").ap()
    cos_t = nc.dram_tensor("cos_t", [128, S], f32, kind="ExternalInput").ap()
    sin_t = nc.dram_tensor("sin_t", [128, S], f32, kind="ExternalInput").ap()
    mswap = nc.dram_tensor("mswap", [128, 128], f32r, kind="ExternalInput").ap()
    zpad = nc.dram_tensor("zpad", [64, S], f32r, kind="ExternalInput").ap()
    ones_in = nc.dram_tensor("ones_in", [1, 64], f32r, kind="ExternalInput").ap()
    out = nc.dram_tensor("out", [S, E], bf16, kind="ExternalOutput").ap()
    if dbg:
        d_qT0 = nc.dram_tensor("d_qT0", [128, S], f32, kind="ExternalOutput").ap()
        d_kT0 = nc.dram_tensor("d_kT0", [128, S], f32, kind="ExternalOutput").ap()
        d_v0 = nc.dram_tensor("d_v0", [128, N_SK * 128], f32, kind="ExternalOutput").ap()
        d_eA = nc.dram_tensor("d_eA", [128, SQ_CHUNK], f32, kind="ExternalOutput").ap()
        d_rb = nc.dram_tensor("d_rb", [128, SQ_CHUNK], f32, kind="ExternalOutput").ap()
        d_ao0 = nc.dram_tensor("d_ao0", [128, S], f32, kind="ExternalOutput").ap()

    EK = E // 128  # 8 contraction tiles over embedding dim

    with tile.TileContext(nc) as tc:
        with (
            tc.tile_pool(name="const", bufs=1) as constp,
            tc.tile_pool(name="qkT", bufs=1) as qkTp,
            tc.tile_pool(name="vsb", bufs=1) as vp,
            tc.tile_pool(name="attnout", bufs=1) as aop,
            tc.tile_pool(name="wo", bufs=1) as wop,
        ):
            msw_sb = constp.tile([128, 128], f32r, tag="msw")
            # ones row placed at partition 64 so its base matches the
            # aug-row operand of the denominator-broadcast matmuls
            onesrow = constp.tile([65, 64], f32r, tag="onesrow")
            ones_f32 = constp.tile([128, N_SK], f32, tag="ones_f32")
            nc.gpsimd.memset(ones_f32[:], 1.0)

            # k' per pair: [128, S] (rows 0:64 head A dims, 64:128 head B).
            # q' per pair is split into two zero-padded [128, S] tensors so the
            # scores matmuls contract over the full K=128 (K=64 f32r matmuls
            # run ~3x slower on the PE): qzlo = [q'_A | 0], qzhi = [0 | q'_B].
            qzlo = [qkTp.tile([128, S], f32r, tag=f"qzlo{g}", name=f"qzlo{g}") for g in range(PAIRS)]
            qzhi = [qkTp.tile([128, S], f32r, tag=f"qzhi{g}", name=f"qzhi{g}") for g in range(PAIRS)]
            kT = [qkTp.tile([128, S], f32r, tag=f"kT{g}", name=f"kT{g}") for g in range(PAIRS)]
            # v natural + aug ones column, 4 heads: head h occupies cols
            # [65h, 65h+64) = v, col 65h+64 = ones (the softmax-denominator row)
            v_c = vp.tile([128, N_SK, 4 * 65], bf16, tag="vc", name="vc")
            for h in range(4):
                nc.vector.tensor_copy(v_c[:, :, 65 * h + 64], ones_f32[:])
            # normalized attention output per pair [128 (pair dims), S]
            att_o = [aop.tile([128, S], f32r, tag=f"ao{g}", name=f"ao{g}") for g in range(PAIRS)]
            # W_out rows per pair
            wo_sb = [wop.tile([128, E], f32r, tag=f"wo{g}", name=f"wo{g}") for g in range(PAIRS)]

            # ---------------- projection + rope (both pairs) ----------------
            with (
                tc.tile_pool(name="xt", bufs=1) as xtp,
                tc.tile_pool(name="wqk", bufs=1) as wqkp,
                tc.tile_pool(name="wv", bufs=1) as wvp,
                tc.tile_pool(name="ropes", bufs=2) as ropep,
                tc.tile_pool(name="trig", bufs=1) as trigp,
                tc.tile_pool(name="projps", bufs=3, space="PSUM") as pjp,
                tc.tile_pool(name="rotps", bufs=3, space="PSUM") as rtp,
                tc.tile_pool(name="vps", bufs=2, space="PSUM") as vpp,
            ):
                cos_sb = trigp.tile([128, S], f32, tag="cos")
                sin_sb = trigp.tile([128, S], f32, tag="sin")
                # DMA order = consumption order: interleave weight tiles with
                # the first xT chunk so the first proj matmul starts early
                # One 3D-AP DMA per tensor-chunk: per-DMA issue costs
                # ~625ns on the DGE ring, so 8 separate e-tile DMAs serialize
                # ~5us of issue time before the first matmul group can start.
                # dram rows 128e+p land at sbuf [p, e, :].
                nc.sync.dma_start(msw_sb[:], mswap[:])
                nc.sync.dma_start(onesrow[64:65, :], ones_in[:])
                wqk_all = wqkp.tile([128, EK, 2 * HPG * D], bf16, tag="wqk")
                xt_all = xtp.tile([128, EK, S], bf16, tag="xt")
                wqk_d = w_qk.rearrange("(ek p) c -> p ek c", p=128)
                xt_d = xT.rearrange("(ek p) s -> p ek s", p=128)
                nc.sync.dma_start(wqk_all[:], wqk_d)
                nc.sync.dma_start(xt_all[:, :, 0:512], xt_d[:, :, 0:512])
                nc.sync.dma_start(cos_sb[:, 0:512], cos_t[:, 0:512])
                nc.sync.dma_start(sin_sb[:, 0:512], sin_t[:, 0:512])
                for c in range(1, 4):
                    csl = slice(512 * c, 512 * (c + 1))
                    nc.sync.dma_start(xt_all[:, :, csl], xt_d[:, :, csl])
                    nc.sync.dma_start(cos_sb[:, csl], cos_t[:, csl])
                    nc.sync.dma_start(sin_sb[:, csl], sin_t[:, csl])
                wv_all = wvp.tile([128, EK, HPG * D], bf16, tag="wv")
                nc.sync.dma_start(
                    wv_all[:], w_v.rearrange("(ek p) c -> p ek c", p=128)
                )
                wqk_sb = [wqk_all[:, e, :] for e in range(EK)]
                xt_sb = [xt_all[:, e, :] for e in range(EK)]
                wv_sb = [wv_all[:, e, :] for e in range(EK)]
                # zero pads are first read by the scores matmuls (~60us in),
                # so they queue after everything the projection needs
                for g in range(PAIRS):
                    nc.sync.dma_start(qzlo[g][64:128, :], zpad[:])
                    nc.sync.dma_start(qzhi[g][0:64, :], zpad[:])
                for g in range(PAIRS):
                    nc.sync.dma_start(wo_sb[g][:], w_o[128 * g : 128 * (g + 1), :])

                rope_pend = []

                def rope_tail():
                    (g_, dest, sl, pp, raw) = rope_pend.pop(0)
                    rp = rtp.tile([128, 512], f32, tag="rot")
                    nc.tensor.matmul(rp[:], msw_sb[:], raw[:], start=True, stop=True)
                    t2 = ropep.tile([128, 512], f32, tag="t2")
                    nc.vector.tensor_mul(t2[:], raw[:], cos_sb[:, sl])
                    t1 = ropep.tile([128, 512], f32, tag="t1")
                    nc.vector.tensor_mul(t1[:], rp[:], sin_sb[:, sl])
                    if dest is None:
                        nc.gpsimd.tensor_tensor(
                            qzlo[g_][0:64, sl], t1[0:64, :], t2[0:64, :],
                            mybir.AluOpType.add,
                        )
                        nc.gpsimd.tensor_tensor(
                            qzhi[g_][64:128, sl], t1[64:128, :], t2[64:128, :],
                            mybir.AluOpType.add,
                        )
                    else:
                        nc.vector.tensor_add(dest[:, sl], t1[:], t2[:])

                for g in range(PAIRS):
                    # --- qT / kT projection + rope, chunked over s ---
                    for ti, dest in ((0, None), (1, kT[g])):
                        coff = ti * HPG * D + 128 * g  # col offset in w_qk
                        for c in range(S // 512):
                            sl = slice(512 * c, 512 * (c + 1))
                            pp = pjp.tile([128, 512], f32, tag="pj")
                            for e in range(EK):
                                nc.tensor.matmul(
                                    pp[:],
                                    wqk_sb[e][:, coff : coff + 128],
                                    xt_sb[e][:, sl],
                                    start=(e == 0),
                                    stop=(e == EK - 1),
                                )
                            raw = ropep.tile([128, 512], f32r, tag="raw")
                            nc.scalar.copy(raw[:], pp[:])
                            rope_pend.append((g, dest, sl, pp, raw))
                            if len(rope_pend) > 1:
                                rope_tail()
                while rope_pend:
                    rope_tail()

                # --- v projection, both pairs at once (N=256) ---
                for st in range(N_SK):
                    vp_ps = vpp.tile([128, 2 * 128], f32, tag="vps")
                    for e in range(EK):
                        nc.tensor.matmul(
                            vp_ps[:],
                            xt_sb[e][:, 128 * st : 128 * (st + 1)],
                            wv_sb[e][:],
                            start=(e == 0),
                            stop=(e == EK - 1),
                        )
                    for h in range(4):
                        nc.vector.tensor_copy(
                            v_c[:, st, 65 * h : 65 * h + 64],
                            vp_ps[:, 64 * h : 64 * h + 64],
                        )
                if dbg:
                    nc.sync.dma_start(d_qT0[:], qT[0][:])
                    nc.sync.dma_start(d_kT0[:], kT[0][:])
                    pass

            # ---------------- attention (both pairs) ----------------
            with (
                tc.tile_pool(name="attps", bufs=1, space="PSUM") as attps,
                tc.tile_pool(name="expp", bufs=4) as expp,
                tc.tile_pool(name="recipp", bufs=2) as rcp,
            ):
                for g in range(PAIRS):
                    for ch in range(N_CH):
                        cslice = slice(SQ_CHUNK * ch, SQ_CHUNK * (ch + 1))
                        oTA = attps.tile([65, SQ_CHUNK], f32, tag="oTA")
                        oTB = attps.tile([65, SQ_CHUNK], f32, tag="oTB")
                        exps = []  # (eA, eB) per sk, attn@v lags one sk
                        hA, hB = 2 * g, 2 * g + 1

                        def attnv(sk):
                            eA, eB = exps[sk]
                            first = sk == 0
                            last = sk == N_SK - 1
                            for n in range(NQ):
                                nsl = slice(512 * n, 512 * (n + 1))
                                nc.tensor.matmul(
                                    oTA[:, nsl],
                                    v_c[:, sk, 65 * hA : 65 * hA + 65],
                                    eA[:, nsl],
                                    start=first,
                                    stop=last,
                                )
                                nc.tensor.matmul(
                                    oTB[:, nsl],
                                    v_c[:, sk, 65 * hB : 65 * hB + 65],
                                    eB[:, nsl],
                                    start=first,
                                    stop=last,
                                )

                        for sk in range(N_SK):
                            sksl = slice(128 * sk, 128 * (sk + 1))
                            sA = attps.tile([128, SQ_CHUNK], f32, tag="sA")
                            sB = attps.tile([128, SQ_CHUNK], f32, tag="sB")
                            # scores, 2 heads row-packed (K=64 each)
                            for n in range(NQ):
                                nsl = slice(512 * n, 512 * (n + 1))
                                gsl = slice(
                                    SQ_CHUNK * ch + 512 * n,
                                    SQ_CHUNK * ch + 512 * (n + 1),
                                )
                                nc.tensor.matmul(
                                    sA[:, nsl],
                                    kT[g][:, sksl],
                                    qzlo[g][:, gsl],
                                    start=True,
                                    stop=True,
                                )
                                nc.tensor.matmul(
                                    sB[:, nsl],
                                    kT[g][:, sksl],
                                    qzhi[g][:, gsl],
                                    start=True,
                                    stop=True,
                                )
                            # exp (scale 1/8 folded in): per sk, ScalarE does
                            # ONE head's exp and the DVE+gpsimd Schraudolph
                            # fast-exp does the other (alternating heads so
                            # the error spreads evenly), halving the ScalarE
                            # stream that paced v1's attention phase.
                            eA = expp.tile([128, SQ_CHUNK], bf16, tag="eA")
                            eB = expp.tile([128, SQ_CHUNK], bf16, tag="eB")
                            s_act, e_act = (sA, eA) if sk % 2 == 0 else (sB, eB)
                            s_sch, e_sch = (sB, eB) if sk % 2 == 0 else (sA, eA)
                            nc.scalar.activation(e_act[:], s_act[:], AF.Exp, scale=0.125)
                            ei = expp.tile([128, SQ_CHUNK], i32, tag="ei", bufs=2)
                            nc.vector.tensor_scalar(
                                ei[:], s_sch[:], SCH_A, SCH_B, ALU.mult, ALU.add
                            )
                            nc.gpsimd.tensor_scalar(
                                e_sch[:], ei.bitcast(f32), 1.0, 0.0, ALU.mult, ALU.add
                            )
                            if dbg and g == 0 and ch == 0 and sk == 0:
                                nc.sync.dma_start(d_eA[:], eA[:])
                            exps.append((eA, eB))
                            # PE heater: one standalone LDWEIGHTS per sk keeps
                            # the PE activity monitor from re-throttling the
                            # clock during exp waits (harmless: every real
                            # matmul self-loads its weights).
                            nc.tensor.ldweights(v_c[:, 0, 0:128])
                            if sk > 1:
                                attnv(sk - 2)
                        attnv(N_SK - 2)
                        attnv(N_SK - 1)
                        # Normalize. Evacuate oTA/oTB to SBUF (f32r) right
                        # away so the next chunk can reuse their PSUM banks.
                        # Aug row 64 holds the denominators: broadcast them
                        # across 64 partitions with a K=1 ones outer-product on
                        # the PE (into psum reusing the sA/sB slots), recip,
                        # then one aligned multiply per head half.
                        oA = rcp.tile([65, SQ_CHUNK], f32r, tag="oA")
                        nc.vector.tensor_copy(oA[:], oTA[:])
                        oB = rcp.tile([65, SQ_CHUNK], f32r, tag="oB")
                        nc.vector.tensor_copy(oB[:], oTB[:])
                        dbA = attps.tile([64, SQ_CHUNK], f32, tag="oTA")
                        dbB = attps.tile([64, SQ_CHUNK], f32, tag="oTB")
                        for n in range(NQ):
                            nsl = slice(512 * n, 512 * (n + 1))
                            nc.tensor.matmul(
                                dbA[:, nsl], onesrow[64:65, :], oA[64:65, nsl],
                                start=True, stop=True,
                            )
                            nc.tensor.matmul(
                                dbB[:, nsl], onesrow[64:65, :], oB[64:65, nsl],
                                start=True, stop=True,
                            )
                        rbA = rcp.tile([64, SQ_CHUNK], f32, tag="rbA")
                        nc.vector.reciprocal_approx_fast(rbA[:], dbA[:])
                        rbB = rcp.tile([64, SQ_CHUNK], f32, tag="rbB")
                        nc.vector.reciprocal_approx_fast(rbB[:], dbB[:])
                        nc.vector.tensor_mul(
                            att_o[g][0:64, cslice], oA[0:64, :], rbA[:]
                        )
                        aoB = rcp.tile([64, SQ_CHUNK], f32r, tag="aoB")
                        nc.vector.tensor_mul(aoB[:], oB[0:64, :], rbB[:])
                        nc.sync.dma_start(att_o[g][64:128, cslice], aoB[:])
                        if dbg and g == 0 and ch == N_CH - 1:
                            nc.sync.dma_start(d_ao0[:], att_o[0][:])

                # ---------------- output projection ----------------
                # (emitted inside the attention pools so the op psum tiles
                # rotate through the sA/sB slots instead of waiting for the
                # whole attention pool to drain)
                with tc.tile_pool(name="osb", bufs=3) as osbp:
                    for st in range(S // 128):
                        ssl = slice(128 * st, 128 * (st + 1))
                        op = attps.tile([128, SQ_CHUNK], f32, tag=("sA", "sB", "oTA", "oTB")[st % 4], name=f"op{st}")
                        for g in range(PAIRS):
                            for n in range(E // 512):
                                nsl = slice(512 * n, 512 * (n + 1))
                                nc.tensor.matmul(
                                    op[:, nsl],
                                    att_o[g][:, ssl],
                                    wo_sb[g][:, nsl],
                                    start=(g == 0),
                                    stop=(g == PAIRS - 1),
                                )
                        ot = osbp.tile([128, E], bf16, tag="ot")
                        if st % 2 == 0:
                            nc.scalar.copy(ot[:], op[:])
                        else:
                            nc.vector.tensor_copy(ot[:], op[:])
                        nc.sync.dma_start(out[ssl, :], ot[:])

    nc.compile()
    return nc


def _get_program():
    if "nc" not in _BUILT:
        _BUILT["nc"] = _build_program()
    return _BUILT["nc"]


def _host_inputs(x, W_qkv, W_out):
    """Build the 8 per-core input maps."""
    import ml_dtypes

    f = np.float32
    bf = ml_dtypes.bfloat16
    x = np.asarray(x, dtype=f)
    W_qkv = np.asarray(W_qkv, dtype=f)
    W_out = np.asarray(W_out, dtype=f)

    inv_freq = 1.0 / (ROPE_THETA ** (np.arange(0, D, 2, dtype=np.float64) / D))
    p = np.arange(128)
    freq_row = inv_freq[(p % D) // 2]  # [128]
    ang = freq_row[:, None] * np.arange(S, dtype=np.float64)[None, :]  # [128, S]
    cos_t = np.cos(ang).astype(f)
    sign = np.where(p % 2 == 0, -1.0, 1.0)[:, None]
    sin_t = (np.sin(ang) * sign).astype(f)

    msw = np.zeros((128, 128), dtype=f)
    msw[p, p ^ 1] = 1.0

    maps = []
    for core in range(N_CORES):
        b, hg = divmod(core, HG)
        hs = [HPG * hg + i for i in range(HPG)]
        w_qk = np.concatenate(
            [W_qkv[:, h * D : (h + 1) * D] for h in hs]
            + [W_qkv[:, ATT + h * D : ATT + (h + 1) * D] for h in hs],
            axis=1,
        )
        w_v = np.concatenate(
            [W_qkv[:, 2 * ATT + h * D : 2 * ATT + (h + 1) * D] for h in hs], axis=1
        )
        w_o = np.concatenate([W_out[h * D : (h + 1) * D, :] for h in hs], axis=0)
        maps.append(
            {
                "xT": np.ascontiguousarray(x[b].T).astype(bf),
                "w_qk": np.ascontiguousarray(w_qk).astype(bf),
                "w_v": np.ascontiguousarray(w_v).astype(bf),
                "w_o": np.ascontiguousarray(w_o),
                "cos_t": cos_t,
                "sin_t": sin_t,
                "mswap": msw,
                "zpad": np.zeros((64, S), dtype=f),
                "ones_in": np.ones((1, 64), dtype=f),
            }
        )
    return maps


def kernel(x, W_qkv, W_out):
    from concourse.bass_utils import run_bass_kernel_spmd

    nc = _get_program()
    maps = _host_inputs(x, W_qkv, W_out)
    res = run_bass_kernel_spmd(nc, maps, core_ids=list(range(N_CORES)))
    out = np.zeros((B, S, E), dtype=np.float32)
    for core in range(N_CORES):
        b = core // HG
        out[b] += np.asarray(res.results[core]["out"], dtype=np.float32)
    return out



# revision 30
# speedup vs baseline: 1.0013x; 1.0013x over previous
"""Trainium2 Bass kernel for MultiHeadSelfAttention with RoPE.

Problem: x[2, 2048, 1024] @ W_qkv[1024, 3072] -> rope(q,k) -> softmax(q k^T/8) v
         -> out @ W_out[1024, 1024].

Sharding (8 cores): batch (2-way) x head-group (4-way, 4 heads each).
Each core computes a partial output [2048, 1024] = attnout_heads @ W_out_rows;
host sums the 4 head-group partials per batch.

All matmul operands use float32r (TF32-like fp32: full-rate on the PE vs 4x
slower for plain fp32, ~1.5e-4 relative error). PSUM accumulation is fp32.

On-core dataflow is fully "transposed" so the PE never needs a transpose:
  qT,kT[c, s] = sum_e W[e, c] * xT[e, s]   (lhsT = W slice, rhs = xT)
  rot = Mswap @ qT (PE), q' = qT*cos + rot*sin_signed (DVE)
  scoresT[sk, sq] = sum_d kT[d, sk] qT[d, sq]  (2 heads row-packed, K=64)
  attnT = exp(scoresT/8) (ScalarE, PSUM->SBUF)
  outT[d, sq] += sum_sk v[sk, d] attnT[sk, sq] (2 heads col-packed, PSUM accum)
  denom[sq]  += sum_sk attnT[sk, sq]           (ones-column matmuls, packed)
  attnout = outT * (1/denom)  -> out_partial[s, e] = attnoutT.T @ W_out_rows
"""

import sys

if "/opt/trn_rl_repo" not in sys.path:
    sys.path.insert(0, "/opt/trn_rl_repo")

import numpy as np

B, S, E = 2, 2048, 1024
ATT = 1024
H = 16
D = 64
HG = 4            # head groups (cores per batch)
HPG = H // HG     # heads per core = 4
PAIRS = HPG // 2  # head pairs per core = 2
ROPE_THETA = 10000.0
N_CORES = 8

SQ_CHUNK = 1024   # sq chunk for exp / attn@v psum tiles
NQ = SQ_CHUNK // 512  # matmuls of N=512 per chunk
N_SK = S // 128   # 16 sk tiles
N_CH = S // SQ_CHUNK  # 2 chunks

# Schraudolph fast-exp: i32 = int(A*s + B); bitcast(i32) ~ exp(0.125*s) with
# +-1.8% rms sawtooth error. C = 482804 was calibrated on this hardware
# (zero mean log error, so fast-exp'd softmax weights are unbiased vs the
# ScalarE ACT-exp'd ones; numerator and denominator use the same values so
# softmax normalization is consistent). Half the exps move off the ScalarE
# critical path onto the otherwise-idle DVE (int math, also releases the
# scores-PSUM WAR early) + gpsimd (bitcast -> bf16 convert).
SCH_A = 0.125 * 12102203.161561485   # 0.125 * 2^23/ln2
SCH_B = 1065353216.0 - 482804.0      # 127*2^23 - C

_BUILT = {}


def _build_program(dbg=False):
    import concourse.bacc as bacc
    import concourse.tile as tile
    import concourse.mybir as mybir

    f32 = mybir.dt.float32
    f32r = mybir.dt.float32r
    bf16 = mybir.dt.bfloat16
    i32 = mybir.dt.int32
    AF = mybir.ActivationFunctionType
    ALU = mybir.AluOpType

    nc = bacc.Bacc(
        "TRN2",
        target_bir_lowering=False,
        debug=False,
        enable_asserts=False,
        num_devices=N_CORES,
    )

    xT = nc.dram_tensor("xT", [E, S], bf16, kind="ExternalInput").ap()
    w_qk = nc.dram_tensor("w_qk", [E, 2 * HPG * D], bf16, kind="ExternalInput").ap()
    w_v = nc.dram_tensor("w_v", [E, HPG * D], bf16, kind="ExternalInput").ap()
    w_o = nc.dram_tensor("w_o", [HPG * D, E], f32r, kind="ExternalInput").ap()
    cos_t = nc.dram_tensor("cos_t", [128, S], f32, kind="ExternalInput").ap()
    sin_t = nc.dram_tensor("sin_t", [128, S], f32, kind="ExternalInput").ap()
    mswap = nc.dram_tensor("mswap", [128, 128], f32r, kind="ExternalInput").ap()
    zpad = nc.dram_tensor("zpad", [64, S], f32r, kind="ExternalInput").ap()
    ones_in = nc.dram_tensor("ones_in", [1, 64], f32r, kind="ExternalInput").ap()
    out = nc.dram_tensor("out", [S, E], bf16, kind="ExternalOutput").ap()
    if dbg:
        d_qT0 = nc.dram_tensor("d_qT0", [128, S], f32, kind="ExternalOutput").ap()
        d_kT0 = nc.dram_tensor("d_kT0", [128, S], f32, kind="ExternalOutput").ap()
        d_v0 = nc.dram_tensor("d_v0", [128, N_SK * 128], f32, kind="ExternalOutput").ap()
        d_eA = nc.dram_tensor("d_eA", [128, SQ_CHUNK], f32, kind="ExternalOutput").ap()
        d_rb = nc.dram_tensor("d_rb", [128, SQ_CHUNK], f32, kind="ExternalOutput").ap()
        d_ao0 = nc.dram_tensor("d_ao0", [128, S], f32, kind="ExternalOutput").ap()

    EK = E // 128  # 8 contraction tiles over embedding dim

    with tile.TileContext(nc) as tc:
        with (
            tc.tile_pool(name="const", bufs=1) as constp,
            tc.tile_pool(name="qkT", bufs=1) as qkTp,
            tc.tile_pool(name="vsb", bufs=1) as vp,
            tc.tile_pool(name="attnout", bufs=1) as aop,
            tc.tile_pool(name="wo", bufs=1) as wop,
        ):
            msw_sb = constp.tile([128, 128], f32r, tag="msw")
            # ones row placed at partition 64 so its base matches the
            # aug-row operand of the denominator-broadcast matmuls
            onesrow = constp.tile([65, 64], f32r, tag="onesrow")
            ones_f32 = constp.tile([128, N_SK], f32, tag="ones_f32")
            nc.gpsimd.memset(ones_f32[:], 1.0)

            # k' per pair: [128, S] (rows 0:64 head A dims, 64:128 head B).
            # q' per pair is split into two zero-padded [128, S] tensors so the
            # scores matmuls contract over the full K=128 (K=64 f32r matmuls
            # run ~3x slower on the PE): qzlo = [q'_A | 0], qzhi = [0 | q'_B].
            qzlo = [qkTp.tile([128, S], f32r, tag=f"qzlo{g}", name=f"qzlo{g}") for g in range(PAIRS)]
            qzhi = [qkTp.tile([128, S], f32r, tag=f"qzhi{g}", name=f"qzhi{g}") for g in range(PAIRS)]
            kT = [qkTp.tile([128, S], f32r, tag=f"kT{g}", name=f"kT{g}") for g in range(PAIRS)]
            # v natural + aug ones column, 4 heads: head h occupies cols
            # [65h, 65h+64) = v, col 65h+64 = ones (the softmax-denominator row)
            v_c = vp.tile([128, N_SK, 4 * 65], bf16, tag="vc", name="vc")
            for h in range(4):
                nc.vector.tensor_copy(v_c[:, :, 65 * h + 64], ones_f32[:])
            # normalized attention output per pair [128 (pair dims), S]
            att_o = [aop.tile([128, S], f32r, tag=f"ao{g}", name=f"ao{g}") for g in range(PAIRS)]
            # W_out rows per pair
            wo_sb = [wop.tile([128, E], f32r, tag=f"wo{g}", name=f"wo{g}") for g in range(PAIRS)]

            # ---------------- projection + rope (both pairs) ----------------
            with (
                tc.tile_pool(name="xt", bufs=1) as xtp,
                tc.tile_pool(name="wqk", bufs=1) as wqkp,
                tc.tile_pool(name="wv", bufs=1) as wvp,
                tc.tile_pool(name="ropes", bufs=2) as ropep,
                tc.tile_pool(name="trig", bufs=1) as trigp,
                tc.tile_pool(name="projps", bufs=3, space="PSUM") as pjp,
                tc.tile_pool(name="rotps", bufs=3, space="PSUM") as rtp,
                tc.tile_pool(name="vps", bufs=2, space="PSUM") as vpp,
            ):
                cos_sb = trigp.tile([128, S], f32, tag="cos")
                sin_sb = trigp.tile([128, S], f32, tag="sin")
                # DMA order = consumption order: interleave weight tiles with
                # the first xT chunk so the first proj matmul starts early
                # One 3D-AP DMA per tensor-chunk: per-DMA issue costs
                # ~625ns on the DGE ring, so 8 separate e-tile DMAs serialize
                # ~5us of issue time before the first matmul group can start.
                # dram rows 128e+p land at sbuf [p, e, :].
                nc.sync.dma_start(msw_sb[:], mswap[:])
                nc.sync.dma_start(onesrow[64:65, :], ones_in[:])
                wqk_all = wqkp.tile([128, EK, 2 * HPG * D], bf16, tag="wqk")
                xt_all = xtp.tile([128, EK, S], bf16, tag="xt")
                wqk_d = w_qk.rearrange("(ek p) c -> p ek c", p=128)
                xt_d = xT.rearrange("(ek p) s -> p ek s", p=128)
                nc.sync.dma_start(wqk_all[:], wqk_d)
                nc.sync.dma_start(xt_all[:, :, 0:512], xt_d[:, :, 0:512])
                nc.sync.dma_start(cos_sb[:, 0:512], cos_t[:, 0:512])
                nc.sync.dma_start(sin_sb[:, 0:512], sin_t[:, 0:512])
                for c in range(1, 4):
                    csl = slice(512 * c, 512 * (c + 1))
                    nc.sync.dma_start(xt_all[:, :, csl], xt_d[:, :, csl])
                    nc.sync.dma_start(cos_sb[:, csl], cos_t[:, csl])
                    nc.sync.dma_start(sin_sb[:, csl], sin_t[:, csl])
                wv_all = wvp.tile([128, EK, HPG * D], bf16, tag="wv")
                nc.sync.dma_start(
                    wv_all[:], w_v.rearrange("(ek p) c -> p ek c", p=128)
                )
                wqk_sb = [wqk_all[:, e, :] for e in range(EK)]
                xt_sb = [xt_all[:, e, :] for e in range(EK)]
                wv_sb = [wv_all[:, e, :] for e in range(EK)]
                # zero pads are first read by the scores matmuls (~60us in),
                # so they queue after everything the projection needs
                for g in range(PAIRS):
                    nc.sync.dma_start(qzlo[g][64:128, :], zpad[:])
                    nc.sync.dma_start(qzhi[g][0:64, :], zpad[:])
                for g in range(PAIRS):
                    nc.sync.dma_start(wo_sb[g][:], w_o[128 * g : 128 * (g + 1), :])

                rope_pend = []

                def rope_tail():
                    (g_, dest, sl, pp, raw) = rope_pend.pop(0)
                    rp = rtp.tile([128, 512], f32, tag="rot")
                    nc.tensor.matmul(rp[:], msw_sb[:], raw[:], start=True, stop=True)
                    t2 = ropep.tile([128, 512], f32, tag="t2")
                    nc.vector.tensor_mul(t2[:], raw[:], cos_sb[:, sl])
                    t1 = ropep.tile([128, 512], f32, tag="t1")
                    nc.vector.tensor_mul(t1[:], rp[:], sin_sb[:, sl])
                    if dest is None:
                        nc.gpsimd.tensor_tensor(
                            qzlo[g_][0:64, sl], t1[0:64, :], t2[0:64, :],
                            mybir.AluOpType.add,
                        )
                        nc.gpsimd.tensor_tensor(
                            qzhi[g_][64:128, sl], t1[64:128, :], t2[64:128, :],
                            mybir.AluOpType.add,
                        )
                    else:
                        nc.vector.tensor_add(dest[:, sl], t1[:], t2[:])

                for g in range(PAIRS):
                    # --- qT / kT projection + rope, chunked over s ---
                    for ti, dest in ((0, None), (1, kT[g])):
                        coff = ti * HPG * D + 128 * g  # col offset in w_qk
                        for c in range(S // 512):
                            sl = slice(512 * c, 512 * (c + 1))
                            pp = pjp.tile([128, 512], f32, tag="pj")
                            for e in range(EK):
                                nc.tensor.matmul(
                                    pp[:],
                                    wqk_sb[e][:, coff : coff + 128],
                                    xt_sb[e][:, sl],
                                    start=(e == 0),
                                    stop=(e == EK - 1),
                                )
                            raw = ropep.tile([128, 512], f32r, tag="raw")
                            nc.scalar.copy(raw[:], pp[:])
                            rope_pend.append((g, dest, sl, pp, raw))
                            if len(rope_pend) > 1:
                                rope_tail()
                while rope_pend:
                    rope_tail()

                # --- v projection, both pairs at once (N=256) ---
                for st in range(N_SK):
                    vp_ps = vpp.tile([128, 2 * 128], f32, tag="vps")
                    for e in range(EK):
                        nc.tensor.matmul(
                            vp_ps[:],
                            xt_sb[e][:, 128 * st : 128 * (st + 1)],
                            wv_sb[e][:],
                            start=(e == 0),
                            stop=(e == EK - 1),
                        )
                    for h in range(4):
                        nc.vector.tensor_copy(
                            v_c[:, st, 65 * h : 65 * h + 64],
                            vp_ps[:, 64 * h : 64 * h + 64],
                        )
                if dbg:
                    nc.sync.dma_start(d_qT0[:], qT[0][:])
                    nc.sync.dma_start(d_kT0[:], kT[0][:])
                    pass

            # ---------------- attention (both pairs) ----------------
            with (
                tc.tile_pool(name="attps", bufs=1, space="PSUM") as attps,
                tc.tile_pool(name="expp", bufs=4) as expp,
                tc.tile_pool(name="recipp", bufs=2) as rcp,
            ):
                for g in range(PAIRS):
                    for ch in range(N_CH):
                        cslice = slice(SQ_CHUNK * ch, SQ_CHUNK * (ch + 1))
                        oTA = attps.tile([65, SQ_CHUNK], f32, tag="oTA")
                        oTB = attps.tile([65, SQ_CHUNK], f32, tag="oTB")
                        exps = []  # (eA, eB) per sk, attn@v lags one sk
                        hA, hB = 2 * g, 2 * g + 1

                        def attnv(sk):
                            eA, eB = exps[sk]
                            first = sk == 0
                            last = sk == N_SK - 1
                            for n in range(NQ):
                                nsl = slice(512 * n, 512 * (n + 1))
                                nc.tensor.matmul(
                                    oTA[:, nsl],
                                    v_c[:, sk, 65 * hA : 65 * hA + 65],
                                    eA[:, nsl],
                                    start=first,
                                    stop=last,
                                )
                                nc.tensor.matmul(
                                    oTB[:, nsl],
                                    v_c[:, sk, 65 * hB : 65 * hB + 65],
                                    eB[:, nsl],
                                    start=first,
                                    stop=last,
                                )

                        for sk in range(N_SK):
                            sksl = slice(128 * sk, 128 * (sk + 1))
                            sA = attps.tile([128, SQ_CHUNK], f32, tag="sA")
                            sB = attps.tile([128, SQ_CHUNK], f32, tag="sB")
                            # scores, 2 heads row-packed (K=64 each)
                            for n in range(NQ):
                                nsl = slice(512 * n, 512 * (n + 1))
                                gsl = slice(
                                    SQ_CHUNK * ch + 512 * n,
                                    SQ_CHUNK * ch + 512 * (n + 1),
                                )
                                nc.tensor.matmul(
                                    sA[:, nsl],
                                    kT[g][:, sksl],
                                    qzlo[g][:, gsl],
                                    start=True,
                                    stop=True,
                                )
                                nc.tensor.matmul(
                                    sB[:, nsl],
                                    kT[g][:, sksl],
                                    qzhi[g][:, gsl],
                                    start=True,
                                    stop=True,
                                )
                            # exp (scale 1/8 folded in): per sk, ScalarE does
                            # ONE head's exp and the DVE+gpsimd Schraudolph
                            # fast-exp does the other (alternating heads so
                            # the error spreads evenly), halving the ScalarE
                            # stream that paced v1's attention phase.
                            eA = expp.tile([128, SQ_CHUNK], bf16, tag="eA")
                            eB = expp.tile([128, SQ_CHUNK], bf16, tag="eB")
                            s_act, e_act = (sA, eA) if sk % 2 == 0 else (sB, eB)
                            s_sch, e_sch = (sB, eB) if sk % 2 == 0 else (sA, eA)
                            nc.scalar.activation(e_act[:], s_act[:], AF.Exp, scale=0.125)
                            ei = expp.tile([128, SQ_CHUNK], i32, tag="ei", bufs=2)
                            nc.vector.tensor_scalar(
                                ei[:], s_sch[:], SCH_A, SCH_B, ALU.mult, ALU.add
                            )
                            nc.gpsimd.tensor_scalar(
                                e_sch[:], ei.bitcast(f32), 1.0, 0.0, ALU.mult, ALU.add
                            )
                            if dbg and g == 0 and ch == 0 and sk == 0:
                                nc.sync.dma_start(d_eA[:], eA[:])
                            exps.append((eA, eB))
                            # PE heater: one standalone LDWEIGHTS per sk keeps
                            # the PE activity monitor from re-throttling the
                            # clock during exp waits (harmless: every real
                            # matmul self-loads its weights).
                            nc.tensor.ldweights(v_c[:, 0, 0:128])
                            # attn@v lags 3 sk: the schraudolph chain
                            # (scores -> DVE int-exp -> gpsimd convert) takes
                            # ~3us, and at lag 2 the attnv weight-loads were
                            # measured stalling ~450ns/sk on the gpsimd sem.
                            if sk > 2:
                                attnv(sk - 3)
                        attnv(N_SK - 3)
                        attnv(N_SK - 2)
                        attnv(N_SK - 1)
                        # Normalize. Evacuate oTA/oTB to SBUF (f32r) right
                        # away so the next chunk can reuse their PSUM banks.
                        # Aug row 64 holds the denominators: broadcast them
                        # across 64 partitions with a K=1 ones outer-product on
                        # the PE (into psum reusing the sA/sB slots), recip,
                        # then one aligned multiply per head half.
                        oA = rcp.tile([65, SQ_CHUNK], f32r, tag="oA")
                        nc.vector.tensor_copy(oA[:], oTA[:])
                        oB = rcp.tile([65, SQ_CHUNK], f32r, tag="oB")
                        nc.vector.tensor_copy(oB[:], oTB[:])
                        dbA = attps.tile([64, SQ_CHUNK], f32, tag="oTA")
                        dbB = attps.tile([64, SQ_CHUNK], f32, tag="oTB")
                        for n in range(NQ):
                            nsl = slice(512 * n, 512 * (n + 1))
                            nc.tensor.matmul(
                                dbA[:, nsl], onesrow[64:65, :], oA[64:65, nsl],
                                start=True, stop=True,
                            )
                            nc.tensor.matmul(
                                dbB[:, nsl], onesrow[64:65, :], oB[64:65, nsl],
                                start=True, stop=True,
                            )
                        rbA = rcp.tile([64, SQ_CHUNK], f32, tag="rbA")
                        nc.vector.reciprocal_approx_fast(rbA[:], dbA[:])
                        rbB = rcp.tile([64, SQ_CHUNK], f32, tag="rbB")
                        nc.vector.reciprocal_approx_fast(rbB[:], dbB[:])
                        nc.vector.tensor_mul(
                            att_o[g][0:64, cslice], oA[0:64, :], rbA[:]
                        )
                        aoB = rcp.tile([64, SQ_CHUNK], f32r, tag="aoB")
                        nc.vector.tensor_mul(aoB[:], oB[0:64, :], rbB[:])
                        nc.sync.dma_start(att_o[g][64:128, cslice], aoB[:])
                        if dbg and g == 0 and ch == N_CH - 1:
                            nc.sync.dma_start(d_ao0[:], att_o[0][:])

                # ---------------- output projection ----------------
                # (emitted inside the attention pools so the op psum tiles
                # rotate through the sA/sB slots instead of waiting for the
                # whole attention pool to drain)
                with tc.tile_pool(name="osb", bufs=3) as osbp:
                    for st in range(S // 128):
                        ssl = slice(128 * st, 128 * (st + 1))
                        op = attps.tile([128, SQ_CHUNK], f32, tag=("sA", "sB", "oTA", "oTB")[st % 4], name=f"op{st}")
                        for g in range(PAIRS):
                            for n in range(E // 512):
                                nsl = slice(512 * n, 512 * (n + 1))
                                nc.tensor.matmul(
                                    op[:, nsl],
                                    att_o[g][:, ssl],
                                    wo_sb[g][:, nsl],
                                    start=(g == 0),
                                    stop=(g == PAIRS - 1),
                                )
                        ot = osbp.tile([128, E], bf16, tag="ot")
                        if st % 2 == 0:
                            nc.scalar.copy(ot[:], op[:])
                        else:
                            nc.vector.tensor_copy(ot[:], op[:])
                        nc.sync.dma_start(out[ssl, :], ot[:])

    nc.compile()
    return nc


def _get_program():
    if "nc" not in _BUILT:
        _BUILT["nc"] = _build_program()
    return _BUILT["nc"]


def _host_inputs(x, W_qkv, W_out):
    """Build the 8 per-core input maps."""
    import ml_dtypes

    f = np.float32
    bf = ml_dtypes.bfloat16
    x = np.asarray(x, dtype=f)
    W_qkv = np.asarray(W_qkv, dtype=f)
    W_out = np.asarray(W_out, dtype=f)

    inv_freq = 1.0 / (ROPE_THETA ** (np.arange(0, D, 2, dtype=np.float64) / D))
    p = np.arange(128)
    freq_row = inv_freq[(p % D) // 2]  # [128]
    ang = freq_row[:, None] * np.arange(S, dtype=np.float64)[None, :]  # [128, S]
    cos_t = np.cos(ang).astype(f)
    sign = np.where(p % 2 == 0, -1.0, 1.0)[:, None]
    sin_t = (np.sin(ang) * sign).astype(f)

    msw = np.zeros((128, 128), dtype=f)
    msw[p, p ^ 1] = 1.0

    maps = []
    for core in range(N_CORES):
        b, hg = divmod(core, HG)
        hs = [HPG * hg + i for i in range(HPG)]
        w_qk = np.concatenate(
            [W_qkv[:, h * D : (h + 1) * D] for h in hs]
            + [W_qkv[:, ATT + h * D : ATT + (h + 1) * D] for h in hs],
            axis=1,
        )
        w_v = np.concatenate(
            [W_qkv[:, 2 * ATT + h * D : 2 * ATT + (h + 1) * D] for h in hs], axis=1
        )
        w_o = np.concatenate([W_out[h * D : (h + 1) * D, :] for h in hs], axis=0)
        maps.append(
            {
                "xT": np.ascontiguousarray(x[b].T).astype(bf),
                "w_qk": np.ascontiguousarray(w_qk).astype(bf),
                "w_v": np.ascontiguousarray(w_v).astype(bf),
                "w_o": np.ascontiguousarray(w_o),
                "cos_t": cos_t,
                "sin_t": sin_t,
                "mswap": msw,
                "zpad": np.zeros((64, S), dtype=f),
                "ones_in": np.ones((1, 64), dtype=f),
            }
        )
    return maps


def kernel(x, W_qkv, W_out):
    from concourse.bass_utils import run_bass_kernel_spmd

    nc = _get_program()
    maps = _host_inputs(x, W_qkv, W_out)
    res = run_bass_kernel_spmd(nc, maps, core_ids=list(range(N_CORES)))
    out = np.zeros((B, S, E), dtype=np.float32)
    for core in range(N_CORES):
        b = core // HG
        out[b] += np.asarray(res.results[core]["out"], dtype=np.float32)
    return out



# revision 32
# speedup vs baseline: 1.0341x; 1.0328x over previous
"""Trainium2 Bass kernel for MultiHeadSelfAttention with RoPE.

Problem: x[2, 2048, 1024] @ W_qkv[1024, 3072] -> rope(q,k) -> softmax(q k^T/8) v
         -> out @ W_out[1024, 1024].

Sharding (8 cores): batch (2-way) x head-group (4-way, 4 heads each).
Each core computes a partial output [2048, 1024] = attnout_heads @ W_out_rows;
host sums the 4 head-group partials per batch.

All matmul operands use float32r (TF32-like fp32: full-rate on the PE vs 4x
slower for plain fp32, ~1.5e-4 relative error). PSUM accumulation is fp32.

On-core dataflow is fully "transposed" so the PE never needs a transpose:
  qT,kT[c, s] = sum_e W[e, c] * xT[e, s]   (lhsT = W slice, rhs = xT)
  rot = Mswap @ qT (PE), q' = qT*cos + rot*sin_signed (DVE)
  scoresT[sk, sq] = sum_d kT[d, sk] qT[d, sq]  (2 heads row-packed, K=64)
  attnT = exp(scoresT/8) (ScalarE, PSUM->SBUF)
  outT[d, sq] += sum_sk v[sk, d] attnT[sk, sq] (2 heads col-packed, PSUM accum)
  denom[sq]  += sum_sk attnT[sk, sq]           (ones-column matmuls, packed)
  attnout = outT * (1/denom)  -> out_partial[s, e] = attnoutT.T @ W_out_rows
"""

import sys

if "/opt/trn_rl_repo" not in sys.path:
    sys.path.insert(0, "/opt/trn_rl_repo")

import numpy as np

B, S, E = 2, 2048, 1024
ATT = 1024
H = 16
D = 64
HG = 4            # head groups (cores per batch)
HPG = H // HG     # heads per core = 4
PAIRS = HPG // 2  # head pairs per core = 2
ROPE_THETA = 10000.0
N_CORES = 8

SQ_CHUNK = 1024   # sq chunk for exp / attn@v psum tiles
NQ = SQ_CHUNK // 512  # matmuls of N=512 per chunk
N_SK = S // 128   # 16 sk tiles
N_CH = S // SQ_CHUNK  # 2 chunks

# Schraudolph fast-exp: i32 = int(A*s + B); bitcast(i32) ~ exp(0.125*s) with
# +-1.8% rms sawtooth error. C = 482804 was calibrated on this hardware
# (zero mean log error, so fast-exp'd softmax weights are unbiased vs the
# ScalarE ACT-exp'd ones; numerator and denominator use the same values so
# softmax normalization is consistent). Half the exps move off the ScalarE
# critical path onto the otherwise-idle DVE (int math, also releases the
# scores-PSUM WAR early) + gpsimd (bitcast -> bf16 convert).
SCH_A = 0.125 * 12102203.161561485   # 0.125 * 2^23/ln2
SCH_B = 1065353216.0 - 482804.0      # 127*2^23 - C

_BUILT = {}


def _build_program(dbg=False):
    import concourse.bacc as bacc
    import concourse.tile as tile
    import concourse.mybir as mybir

    f32 = mybir.dt.float32
    f32r = mybir.dt.float32r
    bf16 = mybir.dt.bfloat16
    i32 = mybir.dt.int32
    AF = mybir.ActivationFunctionType
    ALU = mybir.AluOpType

    nc = bacc.Bacc(
        "TRN2",
        target_bir_lowering=False,
        debug=False,
        enable_asserts=False,
        num_devices=N_CORES,
    )

    xT = nc.dram_tensor("xT", [E, S], bf16, kind="ExternalInput").ap()
    w_qk = nc.dram_tensor("w_qk", [E, 2 * HPG * D], bf16, kind="ExternalInput").ap()
    w_v = nc.dram_tensor("w_v", [E, HPG * D], bf16, kind="ExternalInput").ap()
    w_o = nc.dram_tensor("w_o", [HPG * D, E], f32r, kind="ExternalInput").ap()
    cos_t = nc.dram_tensor("cos_t", [128, S], f32, kind="ExternalInput").ap()
    sin_t = nc.dram_tensor("sin_t", [128, S], f32, kind="ExternalInput").ap()
    mswap = nc.dram_tensor("mswap", [128, 128], f32r, kind="ExternalInput").ap()
    zpad = nc.dram_tensor("zpad", [64, S], f32r, kind="ExternalInput").ap()
    ones_in = nc.dram_tensor("ones_in", [1, 64], f32r, kind="ExternalInput").ap()
    out = nc.dram_tensor("out", [S, E], bf16, kind="ExternalOutput").ap()
    if dbg:
        d_qT0 = nc.dram_tensor("d_qT0", [128, S], f32, kind="ExternalOutput").ap()
        d_kT0 = nc.dram_tensor("d_kT0", [128, S], f32, kind="ExternalOutput").ap()
        d_v0 = nc.dram_tensor("d_v0", [128, N_SK * 128], f32, kind="ExternalOutput").ap()
        d_eA = nc.dram_tensor("d_eA", [128, SQ_CHUNK], f32, kind="ExternalOutput").ap()
        d_rb = nc.dram_tensor("d_rb", [128, SQ_CHUNK], f32, kind="ExternalOutput").ap()
        d_ao0 = nc.dram_tensor("d_ao0", [128, S], f32, kind="ExternalOutput").ap()

    EK = E // 128  # 8 contraction tiles over embedding dim

    with tile.TileContext(nc) as tc:
        with (
            tc.tile_pool(name="const", bufs=1) as constp,
            tc.tile_pool(name="qkT", bufs=1) as qkTp,
            tc.tile_pool(name="vsb", bufs=1) as vp,
            tc.tile_pool(name="attnout", bufs=1) as aop,
            tc.tile_pool(name="wo", bufs=1) as wop,
        ):
            msw_sb = constp.tile([128, 128], f32r, tag="msw")
            # ones row placed at partition 64 so its base matches the
            # aug-row operand of the denominator-broadcast matmuls
            onesrow = constp.tile([65, 64], f32r, tag="onesrow")
            ones_f32 = constp.tile([128, N_SK], f32, tag="ones_f32")
            nc.gpsimd.memset(ones_f32[:], 1.0)

            # k' per pair: [128, S] (rows 0:64 head A dims, 64:128 head B).
            # q' per pair is split into two zero-padded [128, S] tensors so the
            # scores matmuls contract over the full K=128 (K=64 f32r matmuls
            # run ~3x slower on the PE): qzlo = [q'_A | 0], qzhi = [0 | q'_B].
            qzlo = [qkTp.tile([128, S], f32r, tag=f"qzlo{g}", name=f"qzlo{g}") for g in range(PAIRS)]
            qzhi = [qkTp.tile([128, S], f32r, tag=f"qzhi{g}", name=f"qzhi{g}") for g in range(PAIRS)]
            kT = [qkTp.tile([128, S], f32r, tag=f"kT{g}", name=f"kT{g}") for g in range(PAIRS)]
            # v natural + aug ones column, 4 heads: head h occupies cols
            # [65h, 65h+64) = v, col 65h+64 = ones (the softmax-denominator row)
            v_c = vp.tile([128, N_SK, 4 * 65], bf16, tag="vc", name="vc")
            for h in range(4):
                nc.vector.tensor_copy(v_c[:, :, 65 * h + 64], ones_f32[:])
            # normalized attention output per pair [128 (pair dims), S]
            att_o = [aop.tile([128, S], f32r, tag=f"ao{g}", name=f"ao{g}") for g in range(PAIRS)]
            # W_out rows per pair
            wo_sb = [wop.tile([128, E], f32r, tag=f"wo{g}", name=f"wo{g}") for g in range(PAIRS)]

            # ---------------- projection + rope (both pairs) ----------------
            with (
                tc.tile_pool(name="xt", bufs=1) as xtp,
                tc.tile_pool(name="wqk", bufs=1) as wqkp,
                tc.tile_pool(name="wv", bufs=1) as wvp,
                tc.tile_pool(name="ropes", bufs=2) as ropep,
                tc.tile_pool(name="trig", bufs=1) as trigp,
                tc.tile_pool(name="projps", bufs=3, space="PSUM") as pjp,
                tc.tile_pool(name="rotps", bufs=3, space="PSUM") as rtp,
                tc.tile_pool(name="vps", bufs=2, space="PSUM") as vpp,
            ):
                cos_sb = trigp.tile([128, S], f32, tag="cos")
                sin_sb = trigp.tile([128, S], f32, tag="sin")
                # DMA order = consumption order: interleave weight tiles with
                # the first xT chunk so the first proj matmul starts early
                # One 3D-AP DMA per tensor-chunk: per-DMA issue costs
                # ~625ns on the DGE ring, so 8 separate e-tile DMAs serialize
                # ~5us of issue time before the first matmul group can start.
                # dram rows 128e+p land at sbuf [p, e, :].
                nc.sync.dma_start(msw_sb[:], mswap[:])
                nc.sync.dma_start(onesrow[64:65, :], ones_in[:])
                wqk_all = wqkp.tile([128, EK, 2 * HPG * D], bf16, tag="wqk")
                xt_all = xtp.tile([128, EK, S], bf16, tag="xt")
                wqk_d = w_qk.rearrange("(ek p) c -> p ek c", p=128)
                xt_d = xT.rearrange("(ek p) s -> p ek s", p=128)
                nc.sync.dma_start(wqk_all[:], wqk_d)
                nc.sync.dma_start(xt_all[:, :, 0:512], xt_d[:, :, 0:512])
                nc.sync.dma_start(cos_sb[:, 0:512], cos_t[:, 0:512])
                nc.sync.dma_start(sin_sb[:, 0:512], sin_t[:, 0:512])
                for c in range(1, 4):
                    csl = slice(512 * c, 512 * (c + 1))
                    nc.sync.dma_start(xt_all[:, :, csl], xt_d[:, :, csl])
                    nc.sync.dma_start(cos_sb[:, csl], cos_t[:, csl])
                    nc.sync.dma_start(sin_sb[:, csl], sin_t[:, csl])
                wv_all = wvp.tile([128, EK, HPG * D], bf16, tag="wv")
                nc.sync.dma_start(
                    wv_all[:], w_v.rearrange("(ek p) c -> p ek c", p=128)
                )
                wqk_sb = [wqk_all[:, e, :] for e in range(EK)]
                xt_sb = [xt_all[:, e, :] for e in range(EK)]
                wv_sb = [wv_all[:, e, :] for e in range(EK)]
                # zero pads are first read by the scores matmuls (~60us in),
                # so they queue after everything the projection needs
                for g in range(PAIRS):
                    nc.sync.dma_start(qzlo[g][64:128, :], zpad[:])
                    nc.sync.dma_start(qzhi[g][0:64, :], zpad[:])
                for g in range(PAIRS):
                    nc.sync.dma_start(wo_sb[g][:], w_o[128 * g : 128 * (g + 1), :])

                rope_pend = []

                def rope_tail():
                    (g_, dest, sl, pp, raw) = rope_pend.pop(0)
                    rp = rtp.tile([128, 512], f32, tag="rot")
                    nc.tensor.matmul(rp[:], msw_sb[:], raw[:], start=True, stop=True)
                    t2 = ropep.tile([128, 512], f32, tag="t2")
                    nc.vector.tensor_mul(t2[:], raw[:], cos_sb[:, sl])
                    t1 = ropep.tile([128, 512], f32, tag="t1")
                    nc.vector.tensor_mul(t1[:], rp[:], sin_sb[:, sl])
                    if dest is None:
                        nc.gpsimd.tensor_tensor(
                            qzlo[g_][0:64, sl], t1[0:64, :], t2[0:64, :],
                            mybir.AluOpType.add,
                        )
                        nc.gpsimd.tensor_tensor(
                            qzhi[g_][64:128, sl], t1[64:128, :], t2[64:128, :],
                            mybir.AluOpType.add,
                        )
                    else:
                        nc.vector.tensor_add(dest[:, sl], t1[:], t2[:])

                for g in range(PAIRS):
                    # --- qT / kT projection + rope, chunked over s ---
                    for ti, dest in ((0, None), (1, kT[g])):
                        coff = ti * HPG * D + 128 * g  # col offset in w_qk
                        for c in range(S // 512):
                            sl = slice(512 * c, 512 * (c + 1))
                            pp = pjp.tile([128, 512], f32, tag="pj")
                            for e in range(EK):
                                nc.tensor.matmul(
                                    pp[:],
                                    wqk_sb[e][:, coff : coff + 128],
                                    xt_sb[e][:, sl],
                                    start=(e == 0),
                                    stop=(e == EK - 1),
                                )
                            raw = ropep.tile([128, 512], f32r, tag="raw")
                            nc.scalar.copy(raw[:], pp[:])
                            rope_pend.append((g, dest, sl, pp, raw))
                            if len(rope_pend) > 1:
                                rope_tail()
                while rope_pend:
                    rope_tail()

                # --- v projection, both pairs at once (N=256) ---
                for st in range(N_SK):
                    vp_ps = vpp.tile([128, 2 * 128], f32, tag="vps")
                    for e in range(EK):
                        nc.tensor.matmul(
                            vp_ps[:],
                            xt_sb[e][:, 128 * st : 128 * (st + 1)],
                            wv_sb[e][:],
                            start=(e == 0),
                            stop=(e == EK - 1),
                        )
                    for h in range(4):
                        nc.vector.tensor_copy(
                            v_c[:, st, 65 * h : 65 * h + 64],
                            vp_ps[:, 64 * h : 64 * h + 64],
                        )
                if dbg:
                    nc.sync.dma_start(d_qT0[:], qT[0][:])
                    nc.sync.dma_start(d_kT0[:], kT[0][:])
                    pass

            # ---------------- attention (both pairs) ----------------
            with (
                tc.tile_pool(name="attps", bufs=1, space="PSUM") as attps,
                tc.tile_pool(name="expp", bufs=4) as expp,
                tc.tile_pool(name="recipp", bufs=2) as rcp,
            ):
                for g in range(PAIRS):
                    for ch in range(N_CH):
                        cslice = slice(SQ_CHUNK * ch, SQ_CHUNK * (ch + 1))
                        oTA = attps.tile([65, SQ_CHUNK], f32, tag="oTA")
                        oTB = attps.tile([65, SQ_CHUNK], f32, tag="oTB")
                        exps = []  # (eA, eB) per sk, attn@v lags one sk
                        hA, hB = 2 * g, 2 * g + 1

                        def attnv(sk):
                            eA, eB = exps[sk]
                            first = sk == 0
                            last = sk == N_SK - 1
                            for n in range(NQ):
                                nsl = slice(512 * n, 512 * (n + 1))
                                nc.tensor.matmul(
                                    oTA[:, nsl],
                                    v_c[:, sk, 65 * hA : 65 * hA + 65],
                                    eA[:, nsl],
                                    start=first,
                                    stop=last,
                                )
                                nc.tensor.matmul(
                                    oTB[:, nsl],
                                    v_c[:, sk, 65 * hB : 65 * hB + 65],
                                    eB[:, nsl],
                                    start=first,
                                    stop=last,
                                )

                        for sk in range(N_SK):
                            sksl = slice(128 * sk, 128 * (sk + 1))
                            sA = attps.tile([128, SQ_CHUNK], f32, tag="sA")
                            sB = attps.tile([128, SQ_CHUNK], f32, tag="sB")
                            # scores, 2 heads row-packed (K=64 each)
                            for n in range(NQ):
                                nsl = slice(512 * n, 512 * (n + 1))
                                gsl = slice(
                                    SQ_CHUNK * ch + 512 * n,
                                    SQ_CHUNK * ch + 512 * (n + 1),
                                )
                                nc.tensor.matmul(
                                    sA[:, nsl],
                                    kT[g][:, sksl],
                                    qzlo[g][:, gsl],
                                    start=True,
                                    stop=True,
                                )
                                nc.tensor.matmul(
                                    sB[:, nsl],
                                    kT[g][:, sksl],
                                    qzhi[g][:, gsl],
                                    start=True,
                                    stop=True,
                                )
                            # exp (scale 1/8 folded in): per sk, ScalarE does
                            # ONE head's exp and the DVE+gpsimd Schraudolph
                            # fast-exp does the other (alternating heads so
                            # the error spreads evenly), halving the ScalarE
                            # stream that paced v1's attention phase.
                            # head A exp always on ScalarE (so the sA-slot WAR
                            # releases promptly); head B always via the
                            # DVE+gpsimd Schraudolph fast-exp.
                            eA = expp.tile([128, SQ_CHUNK], bf16, tag="eA")
                            eB = expp.tile([128, SQ_CHUNK], bf16, tag="eB")
                            nc.scalar.activation(eA[:], sA[:], AF.Exp, scale=0.125)
                            ei = expp.tile([128, SQ_CHUNK], i32, tag="ei", bufs=2)
                            nc.vector.tensor_scalar(
                                ei[:], sB[:], SCH_A, SCH_B, ALU.mult, ALU.add
                            )
                            nc.gpsimd.tensor_scalar(
                                eB[:], ei.bitcast(f32), 1.0, 0.0, ALU.mult, ALU.add
                            )
                            if dbg and g == 0 and ch == 0 and sk == 0:
                                nc.sync.dma_start(d_eA[:], eA[:])
                            exps.append((eA, eB))
                            # PE heater: one standalone LDWEIGHTS per sk keeps
                            # the PE activity monitor from re-throttling the
                            # clock during exp waits (harmless: every real
                            # matmul self-loads its weights).
                            nc.tensor.ldweights(v_c[:, 0, 0:128])
                            # attn@v lags 3 sk: the schraudolph chain
                            # (scores -> DVE int-exp -> gpsimd convert) takes
                            # ~3us, and at lag 2 the attnv weight-loads were
                            # measured stalling ~450ns/sk on the gpsimd sem.
                            if sk > 2:
                                attnv(sk - 3)
                        attnv(N_SK - 3)
                        attnv(N_SK - 2)
                        attnv(N_SK - 1)
                        # Normalize. Evacuate oTA/oTB to SBUF (f32r) right
                        # away so the next chunk can reuse their PSUM banks.
                        # Aug row 64 holds the denominators: broadcast them
                        # across 64 partitions with a K=1 ones outer-product on
                        # the PE (into psum reusing the sA/sB slots), recip,
                        # then one aligned multiply per head half.
                        oA = rcp.tile([65, SQ_CHUNK], f32r, tag="oA")
                        nc.scalar.copy(oA[:], oTA[:])
                        oB = rcp.tile([65, SQ_CHUNK], f32r, tag="oB")
                        nc.scalar.copy(oB[:], oTB[:])
                        dbA = attps.tile([64, SQ_CHUNK], f32, tag="oTA")
                        dbB = attps.tile([64, SQ_CHUNK], f32, tag="oTB")
                        for n in range(NQ):
                            nsl = slice(512 * n, 512 * (n + 1))
                            nc.tensor.matmul(
                                dbA[:, nsl], onesrow[64:65, :], oA[64:65, nsl],
                                start=True, stop=True,
                            )
                            nc.tensor.matmul(
                                dbB[:, nsl], onesrow[64:65, :], oB[64:65, nsl],
                                start=True, stop=True,
                            )
                        rbA = rcp.tile([64, SQ_CHUNK], f32, tag="rbA")
                        nc.vector.reciprocal_approx_fast(rbA[:], dbA[:])
                        rbB = rcp.tile([64, SQ_CHUNK], f32, tag="rbB")
                        nc.vector.reciprocal_approx_fast(rbB[:], dbB[:])
                        nc.vector.tensor_mul(
                            att_o[g][0:64, cslice], oA[0:64, :], rbA[:]
                        )
                        aoB = rcp.tile([64, SQ_CHUNK], f32r, tag="aoB")
                        nc.vector.tensor_mul(aoB[:], oB[0:64, :], rbB[:])
                        nc.sync.dma_start(att_o[g][64:128, cslice], aoB[:])
                        if dbg and g == 0 and ch == N_CH - 1:
                            nc.sync.dma_start(d_ao0[:], att_o[0][:])

                # ---------------- output projection ----------------
                # (emitted inside the attention pools so the op psum tiles
                # rotate through the sA/sB slots instead of waiting for the
                # whole attention pool to drain)
                with tc.tile_pool(name="osb", bufs=3) as osbp:
                    for st in range(S // 128):
                        ssl = slice(128 * st, 128 * (st + 1))
                        op = attps.tile([128, SQ_CHUNK], f32, tag=("sA", "sB", "oTA", "oTB")[st % 4], name=f"op{st}")
                        for g in range(PAIRS):
                            for n in range(E // 512):
                                nsl = slice(512 * n, 512 * (n + 1))
                                nc.tensor.matmul(
                                    op[:, nsl],
                                    att_o[g][:, ssl],
                                    wo_sb[g][:, nsl],
                                    start=(g == 0),
                                    stop=(g == PAIRS - 1),
                                )
                        ot = osbp.tile([128, E], bf16, tag="ot")
                        if st % 2 == 0:
                            nc.scalar.copy(ot[:], op[:])
                        else:
                            nc.vector.tensor_copy(ot[:], op[:])
                        nc.sync.dma_start(out[ssl, :], ot[:])

    nc.compile()
    return nc


def _get_program():
    if "nc" not in _BUILT:
        _BUILT["nc"] = _build_program()
    return _BUILT["nc"]


def _host_inputs(x, W_qkv, W_out):
    """Build the 8 per-core input maps."""
    import ml_dtypes

    f = np.float32
    bf = ml_dtypes.bfloat16
    x = np.asarray(x, dtype=f)
    W_qkv = np.asarray(W_qkv, dtype=f)
    W_out = np.asarray(W_out, dtype=f)

    inv_freq = 1.0 / (ROPE_THETA ** (np.arange(0, D, 2, dtype=np.float64) / D))
    p = np.arange(128)
    freq_row = inv_freq[(p % D) // 2]  # [128]
    ang = freq_row[:, None] * np.arange(S, dtype=np.float64)[None, :]  # [128, S]
    cos_t = np.cos(ang).astype(f)
    sign = np.where(p % 2 == 0, -1.0, 1.0)[:, None]
    sin_t = (np.sin(ang) * sign).astype(f)

    msw = np.zeros((128, 128), dtype=f)
    msw[p, p ^ 1] = 1.0

    maps = []
    for core in range(N_CORES):
        b, hg = divmod(core, HG)
        hs = [HPG * hg + i for i in range(HPG)]
        w_qk = np.concatenate(
            [W_qkv[:, h * D : (h + 1) * D] for h in hs]
            + [W_qkv[:, ATT + h * D : ATT + (h + 1) * D] for h in hs],
            axis=1,
        )
        w_v = np.concatenate(
            [W_qkv[:, 2 * ATT + h * D : 2 * ATT + (h + 1) * D] for h in hs], axis=1
        )
        w_o = np.concatenate([W_out[h * D : (h + 1) * D, :] for h in hs], axis=0)
        maps.append(
            {
                "xT": np.ascontiguousarray(x[b].T).astype(bf),
                "w_qk": np.ascontiguousarray(w_qk).astype(bf),
                "w_v": np.ascontiguousarray(w_v).astype(bf),
                "w_o": np.ascontiguousarray(w_o),
                "cos_t": cos_t,
                "sin_t": sin_t,
                "mswap": msw,
                "zpad": np.zeros((64, S), dtype=f),
                "ones_in": np.ones((1, 64), dtype=f),
            }
        )
    return maps


def kernel(x, W_qkv, W_out):
    from concourse.bass_utils import run_bass_kernel_spmd

    nc = _get_program()
    maps = _host_inputs(x, W_qkv, W_out)
    res = run_bass_kernel_spmd(nc, maps, core_ids=list(range(N_CORES)))
    out = np.zeros((B, S, E), dtype=np.float32)
    for core in range(N_CORES):
        b = core // HG
        out[b] += np.asarray(res.results[core]["out"], dtype=np.float32)
    return out



# revision 40
# speedup vs baseline: 1.0460x; 1.0115x over previous
"""Trainium2 Bass kernel for MultiHeadSelfAttention with RoPE.

Problem: x[2, 2048, 1024] @ W_qkv[1024, 3072] -> rope(q,k) -> softmax(q k^T/8) v
         -> out @ W_out[1024, 1024].

Sharding (8 cores): batch (2-way) x head-group (4-way, 4 heads each).
Each core computes a partial output [2048, 1024] = attnout_heads @ W_out_rows;
host sums the 4 head-group partials per batch.

All matmul operands use float32r (TF32-like fp32: full-rate on the PE vs 4x
slower for plain fp32, ~1.5e-4 relative error). PSUM accumulation is fp32.

On-core dataflow is fully "transposed" so the PE never needs a transpose:
  qT,kT[c, s] = sum_e W[e, c] * xT[e, s]   (lhsT = W slice, rhs = xT)
  rot = Mswap @ qT (PE), q' = qT*cos + rot*sin_signed (DVE)
  scoresT[sk, sq] = sum_d kT[d, sk] qT[d, sq]  (2 heads row-packed, K=64)
  attnT = exp(scoresT/8) (ScalarE, PSUM->SBUF)
  outT[d, sq] += sum_sk v[sk, d] attnT[sk, sq] (2 heads col-packed, PSUM accum)
  denom[sq]  += sum_sk attnT[sk, sq]           (ones-column matmuls, packed)
  attnout = outT * (1/denom)  -> out_partial[s, e] = attnoutT.T @ W_out_rows
"""

import sys

if "/opt/trn_rl_repo" not in sys.path:
    sys.path.insert(0, "/opt/trn_rl_repo")

import numpy as np

B, S, E = 2, 2048, 1024
ATT = 1024
H = 16
D = 64
HG = 4            # head groups (cores per batch)
HPG = H // HG     # heads per core = 4
PAIRS = HPG // 2  # head pairs per core = 2
ROPE_THETA = 10000.0
N_CORES = 8

SQ_CHUNK = 1024   # sq chunk for exp / attn@v psum tiles
NQ = SQ_CHUNK // 512  # matmuls of N=512 per chunk
N_SK = S // 128   # 16 sk tiles
N_CH = S // SQ_CHUNK  # 2 chunks

# Schraudolph fast-exp: i32 = int(A*s + B); bitcast(i32) ~ exp(0.125*s) with
# +-1.8% rms sawtooth error. C = 482804 was calibrated on this hardware
# (zero mean log error, so fast-exp'd softmax weights are unbiased vs the
# ScalarE ACT-exp'd ones; numerator and denominator use the same values so
# softmax normalization is consistent). Half the exps move off the ScalarE
# critical path onto the otherwise-idle DVE (int math, also releases the
# scores-PSUM WAR early) + gpsimd (bitcast -> bf16 convert).
SCH_A = 0.125 * 12102203.161561485   # 0.125 * 2^23/ln2
SCH_B = 1065353216.0 - 482804.0      # 127*2^23 - C

_BUILT = {}


def _build_program(dbg=False):
    import concourse.bacc as bacc
    import concourse.tile as tile
    import concourse.mybir as mybir

    f32 = mybir.dt.float32
    f32r = mybir.dt.float32r
    bf16 = mybir.dt.bfloat16
    i32 = mybir.dt.int32
    AF = mybir.ActivationFunctionType
    ALU = mybir.AluOpType

    nc = bacc.Bacc(
        "TRN2",
        target_bir_lowering=False,
        debug=False,
        enable_asserts=False,
        num_devices=N_CORES,
    )

    xT = nc.dram_tensor("xT", [E, S], bf16, kind="ExternalInput").ap()
    w_qk = nc.dram_tensor("w_qk", [E, 2 * HPG * D], bf16, kind="ExternalInput").ap()
    w_v = nc.dram_tensor("w_v", [E, HPG * D], bf16, kind="ExternalInput").ap()
    w_o = nc.dram_tensor("w_o", [HPG * D, E], f32r, kind="ExternalInput").ap()
    cos_t = nc.dram_tensor("cos_t", [128, S], f32, kind="ExternalInput").ap()
    sin_t = nc.dram_tensor("sin_t", [128, S], f32, kind="ExternalInput").ap()
    mswap = nc.dram_tensor("mswap", [128, 128], f32r, kind="ExternalInput").ap()
    ones_in = nc.dram_tensor("ones_in", [1, 64], f32r, kind="ExternalInput").ap()
    out = nc.dram_tensor("out", [S, E], bf16, kind="ExternalOutput").ap()
    if dbg:
        d_qT0 = nc.dram_tensor("d_qT0", [128, S], f32, kind="ExternalOutput").ap()
        d_kT0 = nc.dram_tensor("d_kT0", [128, S], f32, kind="ExternalOutput").ap()
        d_v0 = nc.dram_tensor("d_v0", [128, N_SK * 128], f32, kind="ExternalOutput").ap()
        d_eA = nc.dram_tensor("d_eA", [128, SQ_CHUNK], f32, kind="ExternalOutput").ap()
        d_rb = nc.dram_tensor("d_rb", [128, SQ_CHUNK], f32, kind="ExternalOutput").ap()
        d_ao0 = nc.dram_tensor("d_ao0", [128, S], f32, kind="ExternalOutput").ap()

    EK = E // 128  # 8 contraction tiles over embedding dim

    with tile.TileContext(nc) as tc:
        with (
            tc.tile_pool(name="const", bufs=1) as constp,
            tc.tile_pool(name="qkT", bufs=1) as qkTp,
            tc.tile_pool(name="vsb", bufs=1) as vp,
            tc.tile_pool(name="attnout", bufs=1) as aop,
            tc.tile_pool(name="wo", bufs=1) as wop,
        ):
            msw_sb = constp.tile([128, 128], f32r, tag="msw")
            # ones row placed at partition 64 so its base matches the
            # aug-row operand of the denominator-broadcast matmuls
            onesrow = constp.tile([65, 64], f32r, tag="onesrow")
            ones_f32 = constp.tile([128, N_SK], f32, tag="ones_f32")
            nc.gpsimd.memset(ones_f32[:], 1.0)

            # q'/k' per pair: [128, S] bf16, rows 0:64 head A dims, 64:128
            # head B. The scores matmuls are two concurrent row-tiled K=64
            # matmuls (head A on PE array rows 0:63, head B on rows 64:127 via
            # auto tile_position from the base partitions) — ~2x the old
            # zero-padded K=128 scheme.
            qT = [qkTp.tile([128, S], bf16, tag=f"qT{g}", name=f"qT{g}") for g in range(PAIRS)]
            kT = [qkTp.tile([128, S], bf16, tag=f"kT{g}", name=f"kT{g}") for g in range(PAIRS)]
            # v natural + aug ones column, 4 heads: head h occupies cols
            # [65h, 65h+64) = v, col 65h+64 = ones (the softmax-denominator row)
            v_c = vp.tile([128, N_SK, 4 * 65], bf16, tag="vc", name="vc")
            for h in range(4):
                nc.vector.tensor_copy(v_c[:, :, 65 * h + 64], ones_f32[:])
            # normalized attention output per pair [128 (pair dims), S]
            att_o = [aop.tile([128, S], f32r, tag=f"ao{g}", name=f"ao{g}") for g in range(PAIRS)]
            # W_out rows per pair
            wo_sb = [wop.tile([128, E], f32r, tag=f"wo{g}", name=f"wo{g}") for g in range(PAIRS)]

            # ---------------- projection + rope (both pairs) ----------------
            with (
                tc.tile_pool(name="xt", bufs=1) as xtp,
                tc.tile_pool(name="wqk", bufs=1) as wqkp,
                tc.tile_pool(name="wv", bufs=1) as wvp,
                tc.tile_pool(name="ropes", bufs=2) as ropep,
                tc.tile_pool(name="trig", bufs=1) as trigp,
                tc.tile_pool(name="projps", bufs=3, space="PSUM") as pjp,
                tc.tile_pool(name="rotps", bufs=3, space="PSUM") as rtp,
                tc.tile_pool(name="vps", bufs=2, space="PSUM") as vpp,
            ):
                cos_sb = trigp.tile([128, S], f32, tag="cos")
                sin_sb = trigp.tile([128, S], f32, tag="sin")
                # DMA order = consumption order: interleave weight tiles with
                # the first xT chunk so the first proj matmul starts early
                # One 3D-AP DMA per tensor-chunk: per-DMA issue costs
                # ~625ns on the DGE ring, so 8 separate e-tile DMAs serialize
                # ~5us of issue time before the first matmul group can start.
                # dram rows 128e+p land at sbuf [p, e, :].
                nc.sync.dma_start(msw_sb[:], mswap[:])
                nc.sync.dma_start(onesrow[64:65, :], ones_in[:])
                wqk_all = wqkp.tile([128, EK, 2 * HPG * D], bf16, tag="wqk")
                xt_all = xtp.tile([128, EK, S], bf16, tag="xt")
                wqk_d = w_qk.rearrange("(ek p) c -> p ek c", p=128)
                xt_d = xT.rearrange("(ek p) s -> p ek s", p=128)
                nc.sync.dma_start(wqk_all[:], wqk_d)
                nc.sync.dma_start(xt_all[:, :, 0:512], xt_d[:, :, 0:512])
                nc.sync.dma_start(cos_sb[:, 0:512], cos_t[:, 0:512])
                nc.sync.dma_start(sin_sb[:, 0:512], sin_t[:, 0:512])
                for c in range(1, 4):
                    csl = slice(512 * c, 512 * (c + 1))
                    nc.sync.dma_start(xt_all[:, :, csl], xt_d[:, :, csl])
                    nc.sync.dma_start(cos_sb[:, csl], cos_t[:, csl])
                    nc.sync.dma_start(sin_sb[:, csl], sin_t[:, csl])
                wv_all = wvp.tile([128, EK, HPG * D], bf16, tag="wv")
                nc.sync.dma_start(
                    wv_all[:], w_v.rearrange("(ek p) c -> p ek c", p=128)
                )
                wqk_sb = [wqk_all[:, e, :] for e in range(EK)]
                xt_sb = [xt_all[:, e, :] for e in range(EK)]
                wv_sb = [wv_all[:, e, :] for e in range(EK)]
                # zero pads are first read by the scores matmuls (~60us in),
                # so they queue after everything the projection needs
                for g in range(PAIRS):
                    nc.sync.dma_start(wo_sb[g][:], w_o[128 * g : 128 * (g + 1), :])

                rope_pend = []

                def rope_tail():
                    (g_, dest, sl, pp, raw) = rope_pend.pop(0)
                    rp = rtp.tile([128, 512], f32, tag="rot")
                    nc.tensor.matmul(rp[:], msw_sb[:], raw[:], start=True, stop=True)
                    t2 = ropep.tile([128, 512], f32, tag="t2")
                    nc.vector.tensor_mul(t2[:], raw[:], cos_sb[:, sl])
                    t1 = ropep.tile([128, 512], f32, tag="t1")
                    nc.vector.tensor_mul(t1[:], rp[:], sin_sb[:, sl])
                    if g_ == 0:  # q: gpsimd add (spread engine load)
                        nc.gpsimd.tensor_tensor(
                            dest[:, sl], t1[:], t2[:], mybir.AluOpType.add
                        )
                    else:    # k: vector add
                        nc.vector.tensor_add(dest[:, sl], t1[:], t2[:])

                for g in range(PAIRS):
                    # --- qT / kT projection + rope, chunked over s ---
                    for ti, dest in ((0, qT[g]), (1, kT[g])):
                        coff = ti * HPG * D + 128 * g  # col offset in w_qk
                        for c in range(S // 512):
                            sl = slice(512 * c, 512 * (c + 1))
                            pp = pjp.tile([128, 512], f32, tag="pj")
                            for e in range(EK):
                                nc.tensor.matmul(
                                    pp[:],
                                    wqk_sb[e][:, coff : coff + 128],
                                    xt_sb[e][:, sl],
                                    start=(e == 0),
                                    stop=(e == EK - 1),
                                )
                            raw = ropep.tile([128, 512], f32r, tag="raw")
                            nc.scalar.copy(raw[:], pp[:])
                            rope_pend.append((ti, dest, sl, pp, raw))
                            if len(rope_pend) > 1:
                                rope_tail()
                while rope_pend:
                    rope_tail()

                # --- v projection, both pairs at once (N=256) ---
                for st in range(N_SK):
                    vp_ps = vpp.tile([128, 2 * 128], f32, tag="vps")
                    for e in range(EK):
                        nc.tensor.matmul(
                            vp_ps[:],
                            xt_sb[e][:, 128 * st : 128 * (st + 1)],
                            wv_sb[e][:],
                            start=(e == 0),
                            stop=(e == EK - 1),
                        )
                    for h in range(4):
                        nc.vector.tensor_copy(
                            v_c[:, st, 65 * h : 65 * h + 64],
                            vp_ps[:, 64 * h : 64 * h + 64],
                        )
                if dbg:
                    nc.sync.dma_start(d_qT0[:], qT[0][:])
                    nc.sync.dma_start(d_kT0[:], kT[0][:])
                    pass

            # ---------------- attention (both pairs) ----------------
            with (
                tc.tile_pool(name="attps", bufs=1, space="PSUM") as attps,
                tc.tile_pool(name="expp", bufs=4) as expp,
                tc.tile_pool(name="recipp", bufs=2) as rcp,
            ):
                for g in range(PAIRS):
                    for ch in range(N_CH):
                        cslice = slice(SQ_CHUNK * ch, SQ_CHUNK * (ch + 1))
                        oTA = attps.tile([65, SQ_CHUNK], f32, tag="oTA")
                        oTB = attps.tile([65, SQ_CHUNK], f32, tag="oTB")
                        exps = []  # (eA, eB) per sk, attn@v lags one sk
                        hA, hB = 2 * g, 2 * g + 1

                        def attnv(sk):
                            eA, eB = exps[sk]
                            first = sk == 0
                            last = sk == N_SK - 1
                            for n in range(NQ):
                                nsl = slice(512 * n, 512 * (n + 1))
                                nc.tensor.matmul(
                                    oTA[:, nsl],
                                    v_c[:, sk, 65 * hA : 65 * hA + 65],
                                    eA[:, nsl],
                                    start=first,
                                    stop=last,
                                )
                                nc.tensor.matmul(
                                    oTB[:, nsl],
                                    v_c[:, sk, 65 * hB : 65 * hB + 65],
                                    eB[:, nsl],
                                    start=first,
                                    stop=last,
                                )

                        for sk in range(N_SK):
                            sksl = slice(128 * sk, 128 * (sk + 1))
                            sA = attps.tile([128, SQ_CHUNK], f32, tag="sA")
                            sB = attps.tile([128, SQ_CHUNK], f32, tag="sB")
                            # scores: per n-slice, two CONCURRENT row-tiled
                            # K=64 matmuls (head A rows 0:63, head B 64:127)
                            for n in range(NQ):
                                nsl = slice(512 * n, 512 * (n + 1))
                                gsl = slice(
                                    SQ_CHUNK * ch + 512 * n,
                                    SQ_CHUNK * ch + 512 * (n + 1),
                                )
                                nc.tensor.matmul(
                                    sA[:, nsl],
                                    kT[g][0:64, sksl],
                                    qT[g][0:64, gsl],
                                    start=True,
                                    stop=True,
                                )
                                nc.tensor.matmul(
                                    sB[:, nsl],
                                    kT[g][64:128, sksl],
                                    qT[g][64:128, gsl],
                                    start=True,
                                    stop=True,
                                )
                            # exp (scale 1/8 folded in): per sk, ScalarE does
                            # ONE head's exp and the DVE+gpsimd Schraudolph
                            # fast-exp does the other (alternating heads so
                            # the error spreads evenly), halving the ScalarE
                            # stream that paced v1's attention phase.
                            # head A exp always on ScalarE (so the sA-slot WAR
                            # releases promptly); head B always via the
                            # DVE+gpsimd Schraudolph fast-exp.
                            eA = expp.tile([128, SQ_CHUNK], bf16, tag="eA")
                            eB = expp.tile([128, SQ_CHUNK], bf16, tag="eB")
                            # two half-width exps so the sA-slot WAR releases
                            # after ~720ns (subtile deps) instead of 1147ns
                            nc.scalar.activation(
                                eA[:, 0:512], sA[:, 0:512], AF.Exp, scale=0.125
                            )
                            nc.scalar.activation(
                                eA[:, 512:1024], sA[:, 512:1024], AF.Exp, scale=0.125
                            )
                            ei = expp.tile([128, SQ_CHUNK], i32, tag="ei", bufs=2)
                            nc.vector.tensor_scalar(
                                ei[:], sB[:], SCH_A, SCH_B, ALU.mult, ALU.add
                            )
                            nc.gpsimd.tensor_scalar(
                                eB[:], ei.bitcast(f32), 1.0, 0.0, ALU.mult, ALU.add
                            )
                            if dbg and g == 0 and ch == 0 and sk == 0:
                                nc.sync.dma_start(d_eA[:], eA[:])
                            exps.append((eA, eB))
                            # PE heater: one standalone LDWEIGHTS per sk keeps
                            # the PE activity monitor from re-throttling the
                            # clock during exp waits (harmless: every real
                            # matmul self-loads its weights).
                            nc.tensor.ldweights(v_c[:, 0, 0:128])
                            # attn@v lags 3 sk: the schraudolph chain
                            # (scores -> DVE int-exp -> gpsimd convert) takes
                            # ~3us, and at lag 2 the attnv weight-loads were
                            # measured stalling ~450ns/sk on the gpsimd sem.
                            if sk > 2:
                                attnv(sk - 3)
                        attnv(N_SK - 3)
                        attnv(N_SK - 2)
                        attnv(N_SK - 1)
                        # Normalize. Evacuate oTA/oTB to SBUF (f32r) right
                        # away so the next chunk can reuse their PSUM banks.
                        # Aug row 64 holds the denominators: broadcast them
                        # across 64 partitions with a K=1 ones outer-product on
                        # the PE (into psum reusing the sA/sB slots), recip,
                        # then one aligned multiply per head half.
                        oA = rcp.tile([65, SQ_CHUNK], f32r, tag="oA")
                        nc.scalar.copy(oA[:], oTA[:])
                        oB = rcp.tile([65, SQ_CHUNK], f32r, tag="oB")
                        nc.scalar.copy(oB[:], oTB[:])
                        dbA = attps.tile([64, SQ_CHUNK], f32, tag="oTA")
                        dbB = attps.tile([64, SQ_CHUNK], f32, tag="oTB")
                        for n in range(NQ):
                            nsl = slice(512 * n, 512 * (n + 1))
                            nc.tensor.matmul(
                                dbA[:, nsl], onesrow[64:65, :], oA[64:65, nsl],
                                start=True, stop=True,
                            )
                            nc.tensor.matmul(
                                dbB[:, nsl], onesrow[64:65, :], oB[64:65, nsl],
                                start=True, stop=True,
                            )
                        rbA = rcp.tile([64, SQ_CHUNK], f32, tag="rbA")
                        nc.vector.reciprocal_approx_fast(rbA[:], dbA[:])
                        rbB = rcp.tile([64, SQ_CHUNK], f32, tag="rbB")
                        nc.vector.reciprocal_approx_fast(rbB[:], dbB[:])
                        nc.vector.tensor_mul(
                            att_o[g][0:64, cslice], oA[0:64, :], rbA[:]
                        )
                        aoB = rcp.tile([64, SQ_CHUNK], f32r, tag="aoB")
                        nc.vector.tensor_mul(aoB[:], oB[0:64, :], rbB[:])
                        nc.sync.dma_start(att_o[g][64:128, cslice], aoB[:])
                        if dbg and g == 0 and ch == N_CH - 1:
                            nc.sync.dma_start(d_ao0[:], att_o[0][:])

                # ---------------- output projection ----------------
                # (emitted inside the attention pools so the op psum tiles
                # rotate through the sA/sB slots instead of waiting for the
                # whole attention pool to drain)
                with tc.tile_pool(name="osb", bufs=3) as osbp:
                    for st in range(S // 128):
                        ssl = slice(128 * st, 128 * (st + 1))
                        op = attps.tile([128, SQ_CHUNK], f32, tag=("sA", "sB", "oTA", "oTB")[st % 4], name=f"op{st}")
                        for g in range(PAIRS):
                            for n in range(E // 512):
                                nsl = slice(512 * n, 512 * (n + 1))
                                nc.tensor.matmul(
                                    op[:, nsl],
                                    att_o[g][:, ssl],
                                    wo_sb[g][:, nsl],
                                    start=(g == 0),
                                    stop=(g == PAIRS - 1),
                                )
                        ot = osbp.tile([128, E], bf16, tag="ot")
                        if st % 2 == 0:
                            nc.scalar.copy(ot[:], op[:])
                        else:
                            nc.vector.tensor_copy(ot[:], op[:])
                        nc.sync.dma_start(out[ssl, :], ot[:])

    nc.compile()
    return nc


def _get_program():
    if "nc" not in _BUILT:
        _BUILT["nc"] = _build_program()
    return _BUILT["nc"]


def _host_inputs(x, W_qkv, W_out):
    """Build the 8 per-core input maps."""
    import ml_dtypes

    f = np.float32
    bf = ml_dtypes.bfloat16
    x = np.asarray(x, dtype=f)
    W_qkv = np.asarray(W_qkv, dtype=f)
    W_out = np.asarray(W_out, dtype=f)

    inv_freq = 1.0 / (ROPE_THETA ** (np.arange(0, D, 2, dtype=np.float64) / D))
    p = np.arange(128)
    freq_row = inv_freq[(p % D) // 2]  # [128]
    ang = freq_row[:, None] * np.arange(S, dtype=np.float64)[None, :]  # [128, S]
    cos_t = np.cos(ang).astype(f)
    sign = np.where(p % 2 == 0, -1.0, 1.0)[:, None]
    sin_t = (np.sin(ang) * sign).astype(f)

    msw = np.zeros((128, 128), dtype=f)
    msw[p, p ^ 1] = 1.0

    maps = []
    for core in range(N_CORES):
        b, hg = divmod(core, HG)
        hs = [HPG * hg + i for i in range(HPG)]
        w_qk = np.concatenate(
            [W_qkv[:, h * D : (h + 1) * D] for h in hs]
            + [W_qkv[:, ATT + h * D : ATT + (h + 1) * D] for h in hs],
            axis=1,
        )
        w_v = np.concatenate(
            [W_qkv[:, 2 * ATT + h * D : 2 * ATT + (h + 1) * D] for h in hs], axis=1
        )
        w_o = np.concatenate([W_out[h * D : (h + 1) * D, :] for h in hs], axis=0)
        maps.append(
            {
                "xT": np.ascontiguousarray(x[b].T).astype(bf),
                "w_qk": np.ascontiguousarray(w_qk).astype(bf),
                "w_v": np.ascontiguousarray(w_v).astype(bf),
                "w_o": np.ascontiguousarray(w_o),
                "cos_t": cos_t,
                "sin_t": sin_t,
                "mswap": msw,
                "ones_in": np.ones((1, 64), dtype=f),
            }
        )
    return maps


def kernel(x, W_qkv, W_out):
    from concourse.bass_utils import run_bass_kernel_spmd

    nc = _get_program()
    maps = _host_inputs(x, W_qkv, W_out)
    res = run_bass_kernel_spmd(nc, maps, core_ids=list(range(N_CORES)))
    out = np.zeros((B, S, E), dtype=np.float32)
    for core in range(N_CORES):
        b = core // HG
        out[b] += np.asarray(res.results[core]["out"], dtype=np.float32)
    return out



# revision 43
# speedup vs baseline: 1.1030x; 1.0544x over previous
"""Trainium2 Bass kernel for MultiHeadSelfAttention with RoPE.

Problem: x[2, 2048, 1024] @ W_qkv[1024, 3072] -> rope(q,k) -> softmax(q k^T/8) v
         -> out @ W_out[1024, 1024].

Sharding (8 cores): batch (2-way) x head-group (4-way, 4 heads each).
Each core computes a partial output [2048, 1024] = attnout_heads @ W_out_rows;
host sums the 4 head-group partials per batch.

All matmul operands use float32r (TF32-like fp32: full-rate on the PE vs 4x
slower for plain fp32, ~1.5e-4 relative error). PSUM accumulation is fp32.

On-core dataflow is fully "transposed" so the PE never needs a transpose:
  qT,kT[c, s] = sum_e W[e, c] * xT[e, s]   (lhsT = W slice, rhs = xT)
  rot = Mswap @ qT (PE), q' = qT*cos + rot*sin_signed (DVE)
  scoresT[sk, sq] = sum_d kT[d, sk] qT[d, sq]  (2 heads row-packed, K=64)
  attnT = exp(scoresT/8) (ScalarE, PSUM->SBUF)
  outT[d, sq] += sum_sk v[sk, d] attnT[sk, sq] (2 heads col-packed, PSUM accum)
  denom[sq]  += sum_sk attnT[sk, sq]           (ones-column matmuls, packed)
  attnout = outT * (1/denom)  -> out_partial[s, e] = attnoutT.T @ W_out_rows
"""

import sys

if "/opt/trn_rl_repo" not in sys.path:
    sys.path.insert(0, "/opt/trn_rl_repo")

import numpy as np

B, S, E = 2, 2048, 1024
ATT = 1024
H = 16
D = 64
HG = 4            # head groups (cores per batch)
HPG = H // HG     # heads per core = 4
PAIRS = HPG // 2  # head pairs per core = 2
ROPE_THETA = 10000.0
N_CORES = 8

SQ_CHUNK = 512    # sq chunk for exp / attn@v psum tiles
N_SK = S // 128   # 16 sk tiles
N_CH = S // SQ_CHUNK  # 4 chunks

# Schraudolph fast-exp: i32 = int(A*s + B); bitcast(i32) ~ exp(0.125*s) with
# +-1.8% rms sawtooth error, C = 482804 calibrated on this hardware (zero
# mean log error => fast-exp'd softmax weights unbiased vs ACT-exp'd ones;
# numerator and denominator use the same values so softmax stays consistent).
# Head B's exps run on the otherwise-idle DVE (int math; also releases the
# scores-PSUM WAR early) + gpsimd (bitcast -> bf16 convert), halving the
# ScalarE stream.
SCH_A = 0.125 * 12102203.161561485   # 0.125 * 2^23/ln2
SCH_B = 1065353216.0 - 482804.0      # 127*2^23 - C

_BUILT = {}


def _build_program(dbg=False):
    import concourse.bacc as bacc
    import concourse.tile as tile
    import concourse.mybir as mybir

    f32 = mybir.dt.float32
    f32r = mybir.dt.float32r
    bf16 = mybir.dt.bfloat16
    i32 = mybir.dt.int32
    AF = mybir.ActivationFunctionType
    ALU = mybir.AluOpType

    nc = bacc.Bacc(
        "TRN2",
        target_bir_lowering=False,
        debug=False,
        enable_asserts=False,
        num_devices=N_CORES,
    )

    xT = nc.dram_tensor("xT", [E, S], bf16, kind="ExternalInput").ap()
    w_qk = nc.dram_tensor("w_qk", [E, 2 * HPG * D], bf16, kind="ExternalInput").ap()
    w_v = nc.dram_tensor("w_v", [E, HPG * D], bf16, kind="ExternalInput").ap()
    w_o = nc.dram_tensor("w_o", [HPG * D, E], f32r, kind="ExternalInput").ap()
    cos_t = nc.dram_tensor("cos_t", [128, S], f32, kind="ExternalInput").ap()
    sin_t = nc.dram_tensor("sin_t", [128, S], f32, kind="ExternalInput").ap()
    mswap = nc.dram_tensor("mswap", [128, 128], f32r, kind="ExternalInput").ap()
    ones_in = nc.dram_tensor("ones_in", [1, 64], f32r, kind="ExternalInput").ap()
    out = nc.dram_tensor("out", [S, E], bf16, kind="ExternalOutput").ap()
    if dbg:
        d_qT0 = nc.dram_tensor("d_qT0", [128, S], f32, kind="ExternalOutput").ap()
        d_kT0 = nc.dram_tensor("d_kT0", [128, S], f32, kind="ExternalOutput").ap()
        d_v0 = nc.dram_tensor("d_v0", [128, N_SK * 128], f32, kind="ExternalOutput").ap()
        d_eA = nc.dram_tensor("d_eA", [128, SQ_CHUNK], f32, kind="ExternalOutput").ap()
        d_rb = nc.dram_tensor("d_rb", [128, SQ_CHUNK], f32, kind="ExternalOutput").ap()
        d_ao0 = nc.dram_tensor("d_ao0", [128, S], f32, kind="ExternalOutput").ap()

    EK = E // 128  # 8 contraction tiles over embedding dim

    with tile.TileContext(nc) as tc:
        with (
            tc.tile_pool(name="const", bufs=1) as constp,
            tc.tile_pool(name="qkT", bufs=1) as qkTp,
            tc.tile_pool(name="vsb", bufs=1) as vp,
            tc.tile_pool(name="attnout", bufs=1) as aop,
            tc.tile_pool(name="wo", bufs=1) as wop,
        ):
            msw_sb = constp.tile([128, 128], f32r, tag="msw")
            # ones row placed at partition 64 so its base matches the
            # aug-row operand of the denominator-broadcast matmuls
            onesrow = constp.tile([65, 64], f32r, tag="onesrow")
            ones_f32 = constp.tile([128, N_SK], f32, tag="ones_f32")
            nc.gpsimd.memset(ones_f32[:], 1.0)

            # q'/k' per pair: [128, S] bf16, rows 0:64 head A dims, 64:128
            # head B. Scores run as two CONCURRENT row-tiled K=64 matmuls
            # (head A on PE rows 0:63, head B on rows 64:127, auto
            # tile_position from base partitions).
            qT = [qkTp.tile([128, S], bf16, tag=f"qT{g}", name=f"qT{g}") for g in range(PAIRS)]
            kT = [qkTp.tile([128, S], bf16, tag=f"kT{g}", name=f"kT{g}") for g in range(PAIRS)]
            # v natural + aug ones column, 4 heads: head h occupies cols
            # [65h, 65h+64) = v, col 65h+64 = ones (the softmax-denominator row)
            v_c = vp.tile([128, N_SK, 4 * 65], bf16, tag="vc", name="vc")
            for h in range(4):
                nc.vector.tensor_copy(v_c[:, :, 65 * h + 64], ones_f32[:])
            # normalized attention output per pair [128 (pair dims), S]
            att_o = [aop.tile([128, S], f32r, tag=f"ao{g}", name=f"ao{g}") for g in range(PAIRS)]
            # W_out rows per pair
            wo_sb = [wop.tile([128, E], f32r, tag=f"wo{g}", name=f"wo{g}") for g in range(PAIRS)]

            # ---------------- projection + rope (both pairs) ----------------
            with (
                tc.tile_pool(name="xt", bufs=1) as xtp,
                tc.tile_pool(name="wqk", bufs=1) as wqkp,
                tc.tile_pool(name="wv", bufs=1) as wvp,
                tc.tile_pool(name="ropes", bufs=2) as ropep,
                tc.tile_pool(name="trig", bufs=1) as trigp,
                tc.tile_pool(name="projps", bufs=3, space="PSUM") as pjp,
                tc.tile_pool(name="rotps", bufs=3, space="PSUM") as rtp,
                tc.tile_pool(name="vps", bufs=2, space="PSUM") as vpp,
            ):
                cos_sb = trigp.tile([128, S], f32, tag="cos")
                sin_sb = trigp.tile([128, S], f32, tag="sin")
                # DMA order = consumption order: interleave weight tiles with
                # the first xT chunk so the first proj matmul starts early
                # One 3D-AP DMA per tensor-chunk: per-DMA issue costs
                # ~625ns on the DGE ring, so 8 separate e-tile DMAs serialize
                # ~5us of issue time before the first matmul group can start.
                # dram rows 128e+p land at sbuf [p, e, :].
                nc.sync.dma_start(msw_sb[:], mswap[:])
                nc.sync.dma_start(onesrow[64:65, :], ones_in[:])
                wqk_all = wqkp.tile([128, EK, 2 * HPG * D], bf16, tag="wqk")
                xt_all = xtp.tile([128, EK, S], bf16, tag="xt")
                wqk_d = w_qk.rearrange("(ek p) c -> p ek c", p=128)
                xt_d = xT.rearrange("(ek p) s -> p ek s", p=128)
                nc.sync.dma_start(wqk_all[:], wqk_d)
                nc.sync.dma_start(xt_all[:, :, 0:512], xt_d[:, :, 0:512])
                nc.sync.dma_start(cos_sb[:, 0:512], cos_t[:, 0:512])
                nc.sync.dma_start(sin_sb[:, 0:512], sin_t[:, 0:512])
                for c in range(1, 4):
                    csl = slice(512 * c, 512 * (c + 1))
                    nc.sync.dma_start(xt_all[:, :, csl], xt_d[:, :, csl])
                    nc.sync.dma_start(cos_sb[:, csl], cos_t[:, csl])
                    nc.sync.dma_start(sin_sb[:, csl], sin_t[:, csl])
                wv_all = wvp.tile([128, EK, HPG * D], bf16, tag="wv")
                nc.sync.dma_start(
                    wv_all[:], w_v.rearrange("(ek p) c -> p ek c", p=128)
                )
                wqk_sb = [wqk_all[:, e, :] for e in range(EK)]
                xt_sb = [xt_all[:, e, :] for e in range(EK)]
                wv_sb = [wv_all[:, e, :] for e in range(EK)]
                # zero pads are first read by the scores matmuls (~60us in),
                # so they queue after everything the projection needs
                for g in range(PAIRS):
                    nc.sync.dma_start(wo_sb[g][:], w_o[128 * g : 128 * (g + 1), :])

                rope_pend = []

                def rope_tail():
                    (g_, dest, sl, pp, raw) = rope_pend.pop(0)
                    rp = rtp.tile([128, 512], f32, tag="rot")
                    nc.tensor.matmul(rp[:], msw_sb[:], raw[:], start=True, stop=True)
                    t2 = ropep.tile([128, 512], f32, tag="t2")
                    nc.vector.tensor_mul(t2[:], raw[:], cos_sb[:, sl])
                    t1 = ropep.tile([128, 512], f32, tag="t1")
                    nc.vector.tensor_mul(t1[:], rp[:], sin_sb[:, sl])
                    if g_ == 0:  # q: gpsimd add (spread engine load)
                        nc.gpsimd.tensor_tensor(
                            dest[:, sl], t1[:], t2[:], mybir.AluOpType.add
                        )
                    else:    # k: vector add
                        nc.vector.tensor_add(dest[:, sl], t1[:], t2[:])

                for g in range(PAIRS):
                    # --- qT / kT projection + rope, chunked over s ---
                    for ti, dest in ((0, qT[g]), (1, kT[g])):
                        coff = ti * HPG * D + 128 * g  # col offset in w_qk
                        for c in range(S // 512):
                            sl = slice(512 * c, 512 * (c + 1))
                            pp = pjp.tile([128, 512], f32, tag="pj")
                            for e in range(EK):
                                nc.tensor.matmul(
                                    pp[:],
                                    wqk_sb[e][:, coff : coff + 128],
                                    xt_sb[e][:, sl],
                                    start=(e == 0),
                                    stop=(e == EK - 1),
                                )
                            raw = ropep.tile([128, 512], f32r, tag="raw")
                            nc.scalar.copy(raw[:], pp[:])
                            rope_pend.append((ti, dest, sl, pp, raw))
                            if len(rope_pend) > 1:
                                rope_tail()
                while rope_pend:
                    rope_tail()

                # --- v projection, both pairs at once (N=256) ---
                for st in range(N_SK):
                    vp_ps = vpp.tile([128, 2 * 128], f32, tag="vps")
                    for e in range(EK):
                        nc.tensor.matmul(
                            vp_ps[:],
                            xt_sb[e][:, 128 * st : 128 * (st + 1)],
                            wv_sb[e][:],
                            start=(e == 0),
                            stop=(e == EK - 1),
                        )
                    for h in range(4):
                        nc.vector.tensor_copy(
                            v_c[:, st, 65 * h : 65 * h + 64],
                            vp_ps[:, 64 * h : 64 * h + 64],
                        )
                if dbg:
                    nc.sync.dma_start(d_qT0[:], qT[0][:])
                    nc.sync.dma_start(d_kT0[:], kT[0][:])
                    pass

            # ---------------- attention (both pairs) ----------------
            with (
                tc.tile_pool(name="attps", bufs=1, space="PSUM") as attps,
                tc.tile_pool(name="expp", bufs=4) as expp,
                tc.tile_pool(name="recipp", bufs=2) as rcp,
            ):
                for g in range(PAIRS):
                    for ch in range(N_CH):
                        cslice = slice(SQ_CHUNK * ch, SQ_CHUNK * (ch + 1))
                        # attn@v accumulators, row-tiled: per head, the K=128
                        # sk contraction splits into two CONCURRENT K=64
                        # matmuls (v rows 0:64 / 64:128 via auto tile_position)
                        # accumulating into separate single-bank PSUM tiles,
                        # summed once at normalization time.
                        oT = {
                            (h, half): attps.tile(
                                [65, SQ_CHUNK], f32, tag=f"oT{h}{half}",
                                name=f"oT{h}{half}",
                            )
                            for h in range(2)
                            for half in range(2)
                        }
                        exps = []  # (eA, eB) per sk, attn@v lags three sk
                        hA, hB = 2 * g, 2 * g + 1

                        def attnv(sk):
                            eA, eB = exps[sk]
                            first = sk == 0
                            last = sk == N_SK - 1
                            for h, e_t in ((0, eA), (1, eB)):
                                hh = 2 * g + h
                                for half in range(2):
                                    psl = slice(64 * half, 64 * half + 64)
                                    nc.tensor.matmul(
                                        oT[(h, half)][:],
                                        v_c[psl, sk, 65 * hh : 65 * hh + 65],
                                        e_t[psl, :],
                                        start=first,
                                        stop=last,
                                    )

                        for sk in range(N_SK):
                            sksl = slice(128 * sk, 128 * (sk + 1))
                            # depth-2 ping-pong on single-bank score slots
                            sA = attps.tile([128, SQ_CHUNK], f32, tag=f"sA{sk % 2}", name="sA")
                            sB = attps.tile([128, SQ_CHUNK], f32, tag=f"sB{sk % 2}", name="sB")
                            # scores: two CONCURRENT row-tiled K=64 matmuls
                            nc.tensor.matmul(
                                sA[:],
                                kT[g][0:64, sksl],
                                qT[g][0:64, cslice],
                                start=True,
                                stop=True,
                            )
                            nc.tensor.matmul(
                                sB[:],
                                kT[g][64:128, sksl],
                                qT[g][64:128, cslice],
                                start=True,
                                stop=True,
                            )
                            # head A exp on ScalarE; head B via Schraudolph
                            # fast-exp (DVE int math + gpsimd bitcast->bf16)
                            eA = expp.tile([128, SQ_CHUNK], bf16, tag="eA")
                            nc.scalar.activation(eA[:], sA[:], AF.Exp, scale=0.125)
                            ei = expp.tile([128, SQ_CHUNK], i32, tag="ei", bufs=2)
                            nc.vector.tensor_scalar(
                                ei[:], sB[:], SCH_A, SCH_B, ALU.mult, ALU.add
                            )
                            eB = expp.tile([128, SQ_CHUNK], bf16, tag="eB")
                            nc.gpsimd.tensor_scalar(
                                eB[:], ei.bitcast(f32), 1.0, 0.0, ALU.mult, ALU.add
                            )
                            exps.append((eA, eB))
                            # PE heater: standalone LDWEIGHTS keeps the PE
                            # activity monitor from re-throttling the clock
                            # during producer waits (every real matmul
                            # self-loads its weights, so this is harmless).
                            nc.tensor.ldweights(v_c[:, 0, 0:128])
                            # attn@v lags 3 sk so the scores->DVE->gpsimd
                            # chain (~3us) is always complete
                            if sk > 2:
                                attnv(sk - 3)
                        attnv(N_SK - 3)
                        attnv(N_SK - 2)
                        attnv(N_SK - 1)
                        # Normalize. Sum the row-tiled halves (hi half evac'd
                        # by ScalarE, DVE adds PSUM+SBUF), then broadcast the
                        # aug-row denominators with a K=1 ones outer-product,
                        # reciprocal, and one aligned multiply per head.
                        oS = {}
                        for h in range(2):
                            ohi = rcp.tile([65, SQ_CHUNK], f32r, tag=f"ohi{h}", name="ohi")
                            nc.scalar.copy(ohi[:], oT[(h, 1)][:])
                            o_s = rcp.tile([65, SQ_CHUNK], f32r, tag=f"o{h}", name="os")
                            nc.vector.tensor_add(o_s[:], oT[(h, 0)][:], ohi[:])
                            oS[h] = o_s
                        for h in range(2):
                            o_s = oS[h]
                            db = attps.tile([64, SQ_CHUNK], f32, tag=f"oT{h}0", name="db")
                            nc.tensor.matmul(
                                db[:], onesrow[64:65, :], o_s[64:65, :],
                                start=True, stop=True,
                            )
                            rb = rcp.tile([64, SQ_CHUNK], f32, tag=f"rb{h}", name="rb")
                            nc.vector.reciprocal_approx_fast(rb[:], db[:])
                            if h == 0:
                                nc.vector.tensor_mul(
                                    att_o[g][0:64, cslice], o_s[0:64, :], rb[:]
                                )
                            else:
                                aoB = rcp.tile([64, SQ_CHUNK], f32r, tag="aoB", name="aoB")
                                nc.vector.tensor_mul(aoB[:], o_s[0:64, :], rb[:])
                                nc.sync.dma_start(att_o[g][64:128, cslice], aoB[:])

                # ---------------- output projection ----------------
                # (emitted inside the attention pools so the op psum tiles
                # rotate through the score slots instead of waiting for the
                # whole attention pool to drain)
                with tc.tile_pool(name="osb", bufs=3) as osbp:
                    for st in range(S // 128):
                        ssl = slice(128 * st, 128 * (st + 1))
                        ot = osbp.tile([128, E], bf16, tag="ot")
                        for n in range(E // 512):
                            nsl = slice(512 * n, 512 * (n + 1))
                            op = attps.tile(
                                [128, 512], f32,
                                tag=("sA0", "sA1", "sB0", "sB1")[(2 * st + n) % 4],
                                name=f"op{st}_{n}",
                            )
                            for g in range(PAIRS):
                                nc.tensor.matmul(
                                    op[:],
                                    att_o[g][:, ssl],
                                    wo_sb[g][:, nsl],
                                    start=(g == 0),
                                    stop=(g == PAIRS - 1),
                                )
                            if (st + n) % 2 == 0:
                                nc.scalar.copy(ot[:, nsl], op[:])
                            else:
                                nc.vector.tensor_copy(ot[:, nsl], op[:])
                        nc.sync.dma_start(out[ssl, :], ot[:])

    nc.compile()
    return nc


def _get_program():
    if "nc" not in _BUILT:
        _BUILT["nc"] = _build_program()
    return _BUILT["nc"]


def _host_inputs(x, W_qkv, W_out):
    """Build the 8 per-core input maps."""
    import ml_dtypes

    f = np.float32
    bf = ml_dtypes.bfloat16
    x = np.asarray(x, dtype=f)
    W_qkv = np.asarray(W_qkv, dtype=f)
    W_out = np.asarray(W_out, dtype=f)

    inv_freq = 1.0 / (ROPE_THETA ** (np.arange(0, D, 2, dtype=np.float64) / D))
    p = np.arange(128)
    freq_row = inv_freq[(p % D) // 2]  # [128]
    ang = freq_row[:, None] * np.arange(S, dtype=np.float64)[None, :]  # [128, S]
    cos_t = np.cos(ang).astype(f)
    sign = np.where(p % 2 == 0, -1.0, 1.0)[:, None]
    sin_t = (np.sin(ang) * sign).astype(f)

    msw = np.zeros((128, 128), dtype=f)
    msw[p, p ^ 1] = 1.0

    maps = []
    for core in range(N_CORES):
        b, hg = divmod(core, HG)
        hs = [HPG * hg + i for i in range(HPG)]
        w_qk = np.concatenate(
            [W_qkv[:, h * D : (h + 1) * D] for h in hs]
            + [W_qkv[:, ATT + h * D : ATT + (h + 1) * D] for h in hs],
            axis=1,
        )
        w_v = np.concatenate(
            [W_qkv[:, 2 * ATT + h * D : 2 * ATT + (h + 1) * D] for h in hs], axis=1
        )
        w_o = np.concatenate([W_out[h * D : (h + 1) * D, :] for h in hs], axis=0)
        maps.append(
            {
                "xT": np.ascontiguousarray(x[b].T).astype(bf),
                "w_qk": np.ascontiguousarray(w_qk).astype(bf),
                "w_v": np.ascontiguousarray(w_v).astype(bf),
                "w_o": np.ascontiguousarray(w_o),
                "cos_t": cos_t,
                "sin_t": sin_t,
                "mswap": msw,
                "ones_in": np.ones((1, 64), dtype=f),
            }
        )
    return maps


def kernel(x, W_qkv, W_out):
    from concourse.bass_utils import run_bass_kernel_spmd

    nc = _get_program()
    maps = _host_inputs(x, W_qkv, W_out)
    res = run_bass_kernel_spmd(nc, maps, core_ids=list(range(N_CORES)))
    out = np.zeros((B, S, E), dtype=np.float32)
    for core in range(N_CORES):
        b = core // HG
        out[b] += np.asarray(res.results[core]["out"], dtype=np.float32)
    return out



# revision 44
# speedup vs baseline: 1.1523x; 1.0447x over previous
"""Trainium2 Bass kernel for MultiHeadSelfAttention with RoPE.

Problem: x[2, 2048, 1024] @ W_qkv[1024, 3072] -> rope(q,k) -> softmax(q k^T/8) v
         -> out @ W_out[1024, 1024].

Sharding (8 cores): batch (2-way) x head-group (4-way, 4 heads each).
Each core computes a partial output [2048, 1024] = attnout_heads @ W_out_rows;
host sums the 4 head-group partials per batch.

All matmul operands use float32r (TF32-like fp32: full-rate on the PE vs 4x
slower for plain fp32, ~1.5e-4 relative error). PSUM accumulation is fp32.

On-core dataflow is fully "transposed" so the PE never needs a transpose:
  qT,kT[c, s] = sum_e W[e, c] * xT[e, s]   (lhsT = W slice, rhs = xT)
  rot = Mswap @ qT (PE), q' = qT*cos + rot*sin_signed (DVE)
  scoresT[sk, sq] = sum_d kT[d, sk] qT[d, sq]  (2 heads row-packed, K=64)
  attnT = exp(scoresT/8) (ScalarE, PSUM->SBUF)
  outT[d, sq] += sum_sk v[sk, d] attnT[sk, sq] (2 heads col-packed, PSUM accum)
  denom[sq]  += sum_sk attnT[sk, sq]           (ones-column matmuls, packed)
  attnout = outT * (1/denom)  -> out_partial[s, e] = attnoutT.T @ W_out_rows
"""

import sys

if "/opt/trn_rl_repo" not in sys.path:
    sys.path.insert(0, "/opt/trn_rl_repo")

import numpy as np

B, S, E = 2, 2048, 1024
ATT = 1024
H = 16
D = 64
HG = 4            # head groups (cores per batch)
HPG = H // HG     # heads per core = 4
PAIRS = HPG // 2  # head pairs per core = 2
ROPE_THETA = 10000.0
N_CORES = 8

SQ_CHUNK = 512    # sq chunk for exp / attn@v psum tiles
N_SK = S // 128   # 16 sk tiles
N_CH = S // SQ_CHUNK  # 4 chunks

# Schraudolph fast-exp: i32 = int(A*s + B); bitcast(i32) ~ exp(0.125*s) with
# +-1.8% rms sawtooth error, C = 482804 calibrated on this hardware (zero
# mean log error => fast-exp'd softmax weights unbiased vs ACT-exp'd ones;
# numerator and denominator use the same values so softmax stays consistent).
# Head B's exps run on the otherwise-idle DVE (int math; also releases the
# scores-PSUM WAR early) + gpsimd (bitcast -> bf16 convert), halving the
# ScalarE stream.
SCH_A = 0.125 * 12102203.161561485   # 0.125 * 2^23/ln2
SCH_B = 1065353216.0 - 482804.0      # 127*2^23 - C

_BUILT = {}


def _build_program(dbg=False):
    import concourse.bacc as bacc
    import concourse.tile as tile
    import concourse.mybir as mybir

    f32 = mybir.dt.float32
    f32r = mybir.dt.float32r
    bf16 = mybir.dt.bfloat16
    i32 = mybir.dt.int32
    AF = mybir.ActivationFunctionType
    ALU = mybir.AluOpType

    nc = bacc.Bacc(
        "TRN2",
        target_bir_lowering=False,
        debug=False,
        enable_asserts=False,
        num_devices=N_CORES,
    )

    xT = nc.dram_tensor("xT", [E, S], bf16, kind="ExternalInput").ap()
    w_qk = nc.dram_tensor("w_qk", [E, 2 * HPG * D], bf16, kind="ExternalInput").ap()
    w_v = nc.dram_tensor("w_v", [E, HPG * D], bf16, kind="ExternalInput").ap()
    w_o = nc.dram_tensor("w_o", [HPG * D, E], f32r, kind="ExternalInput").ap()
    cos_t = nc.dram_tensor("cos_t", [128, S], f32, kind="ExternalInput").ap()
    sin_t = nc.dram_tensor("sin_t", [128, S], f32, kind="ExternalInput").ap()
    mswap = nc.dram_tensor("mswap", [128, 128], f32r, kind="ExternalInput").ap()
    ones_in = nc.dram_tensor("ones_in", [1, 64], f32r, kind="ExternalInput").ap()
    out = nc.dram_tensor("out", [S, E], bf16, kind="ExternalOutput").ap()
    if dbg:
        d_qT0 = nc.dram_tensor("d_qT0", [128, S], f32, kind="ExternalOutput").ap()
        d_kT0 = nc.dram_tensor("d_kT0", [128, S], f32, kind="ExternalOutput").ap()
        d_v0 = nc.dram_tensor("d_v0", [128, N_SK * 128], f32, kind="ExternalOutput").ap()
        d_eA = nc.dram_tensor("d_eA", [128, SQ_CHUNK], f32, kind="ExternalOutput").ap()
        d_rb = nc.dram_tensor("d_rb", [128, SQ_CHUNK], f32, kind="ExternalOutput").ap()
        d_ao0 = nc.dram_tensor("d_ao0", [128, S], f32, kind="ExternalOutput").ap()

    EK = E // 128  # 8 contraction tiles over embedding dim

    with tile.TileContext(nc) as tc:
        with (
            tc.tile_pool(name="const", bufs=1) as constp,
            tc.tile_pool(name="qkT", bufs=1) as qkTp,
            tc.tile_pool(name="vsb", bufs=1) as vp,
            tc.tile_pool(name="attnout", bufs=1) as aop,
            tc.tile_pool(name="wo", bufs=1) as wop,
        ):
            msw_sb = constp.tile([128, 128], f32r, tag="msw")
            # ones row placed at partition 64 so its base matches the
            # aug-row operand of the denominator-broadcast matmuls
            onesrow = constp.tile([65, 64], f32r, tag="onesrow")
            ones_f32 = constp.tile([128, N_SK], f32, tag="ones_f32")
            nc.gpsimd.memset(ones_f32[:], 1.0)

            # q'/k' per pair: [128, S] bf16, rows 0:64 head A dims, 64:128
            # head B. Scores run as two CONCURRENT row-tiled K=64 matmuls
            # (head A on PE rows 0:63, head B on rows 64:127, auto
            # tile_position from base partitions).
            qT = [qkTp.tile([128, S], bf16, tag=f"qT{g}", name=f"qT{g}") for g in range(PAIRS)]
            kT = [qkTp.tile([128, S], bf16, tag=f"kT{g}", name=f"kT{g}") for g in range(PAIRS)]
            # v natural + aug ones column, 4 heads: head h occupies cols
            # [65h, 65h+64) = v, col 65h+64 = ones (the softmax-denominator row)
            v_c = vp.tile([128, N_SK, 4 * 65], bf16, tag="vc", name="vc")
            for h in range(4):
                nc.vector.tensor_copy(v_c[:, :, 65 * h + 64], ones_f32[:])
            # normalized attention output per pair [128 (pair dims), S]
            att_o = [aop.tile([128, S], f32r, tag=f"ao{g}", name=f"ao{g}") for g in range(PAIRS)]
            # W_out rows per pair
            wo_sb = [wop.tile([128, E], f32r, tag=f"wo{g}", name=f"wo{g}") for g in range(PAIRS)]

            # ---------------- projection + rope (both pairs) ----------------
            with (
                tc.tile_pool(name="xt", bufs=1) as xtp,
                tc.tile_pool(name="wqk", bufs=1) as wqkp,
                tc.tile_pool(name="wv", bufs=1) as wvp,
                tc.tile_pool(name="ropes", bufs=2) as ropep,
                tc.tile_pool(name="trig", bufs=1) as trigp,
                tc.tile_pool(name="projps", bufs=3, space="PSUM") as pjp,
                tc.tile_pool(name="rotps", bufs=3, space="PSUM") as rtp,
                tc.tile_pool(name="vps", bufs=2, space="PSUM") as vpp,
            ):
                cos_sb = trigp.tile([128, S], f32, tag="cos")
                sin_sb = trigp.tile([128, S], f32, tag="sin")
                # DMA order = consumption order: interleave weight tiles with
                # the first xT chunk so the first proj matmul starts early
                # One 3D-AP DMA per tensor-chunk: per-DMA issue costs
                # ~625ns on the DGE ring, so 8 separate e-tile DMAs serialize
                # ~5us of issue time before the first matmul group can start.
                # dram rows 128e+p land at sbuf [p, e, :].
                nc.sync.dma_start(msw_sb[:], mswap[:])
                nc.sync.dma_start(onesrow[64:65, :], ones_in[:])
                wqk_all = wqkp.tile([128, EK, 2 * HPG * D], bf16, tag="wqk")
                xt_all = xtp.tile([128, EK, S], bf16, tag="xt")
                wqk_d = w_qk.rearrange("(ek p) c -> p ek c", p=128)
                xt_d = xT.rearrange("(ek p) s -> p ek s", p=128)
                nc.sync.dma_start(wqk_all[:], wqk_d)
                nc.sync.dma_start(xt_all[:, :, 0:512], xt_d[:, :, 0:512])
                nc.sync.dma_start(cos_sb[:, 0:512], cos_t[:, 0:512])
                nc.sync.dma_start(sin_sb[:, 0:512], sin_t[:, 0:512])
                for c in range(1, 4):
                    csl = slice(512 * c, 512 * (c + 1))
                    nc.sync.dma_start(xt_all[:, :, csl], xt_d[:, :, csl])
                    nc.sync.dma_start(cos_sb[:, csl], cos_t[:, csl])
                    nc.sync.dma_start(sin_sb[:, csl], sin_t[:, csl])
                wv_all = wvp.tile([128, EK, HPG * D], bf16, tag="wv")
                nc.sync.dma_start(
                    wv_all[:], w_v.rearrange("(ek p) c -> p ek c", p=128)
                )
                wqk_sb = [wqk_all[:, e, :] for e in range(EK)]
                xt_sb = [xt_all[:, e, :] for e in range(EK)]
                wv_sb = [wv_all[:, e, :] for e in range(EK)]
                # zero pads are first read by the scores matmuls (~60us in),
                # so they queue after everything the projection needs
                for g in range(PAIRS):
                    nc.sync.dma_start(wo_sb[g][:], w_o[128 * g : 128 * (g + 1), :])

                rope_pend = []

                def rope_tail():
                    (g_, dest, sl, pp, raw) = rope_pend.pop(0)
                    rp = rtp.tile([128, 512], f32, tag="rot")
                    nc.tensor.matmul(rp[:], msw_sb[:], raw[:], start=True, stop=True)
                    t2 = ropep.tile([128, 512], f32, tag="t2")
                    nc.vector.tensor_mul(t2[:], raw[:], cos_sb[:, sl])
                    t1 = ropep.tile([128, 512], f32, tag="t1")
                    nc.vector.tensor_mul(t1[:], rp[:], sin_sb[:, sl])
                    if g_ == 0:  # q: gpsimd add (spread engine load)
                        nc.gpsimd.tensor_tensor(
                            dest[:, sl], t1[:], t2[:], mybir.AluOpType.add
                        )
                    else:    # k: vector add
                        nc.vector.tensor_add(dest[:, sl], t1[:], t2[:])

                for g in range(PAIRS):
                    # --- qT / kT projection + rope, chunked over s ---
                    for ti, dest in ((0, qT[g]), (1, kT[g])):
                        coff = ti * HPG * D + 128 * g  # col offset in w_qk
                        for c in range(S // 512):
                            sl = slice(512 * c, 512 * (c + 1))
                            pp = pjp.tile([128, 512], f32, tag="pj")
                            for e in range(EK):
                                nc.tensor.matmul(
                                    pp[:],
                                    wqk_sb[e][:, coff : coff + 128],
                                    xt_sb[e][:, sl],
                                    start=(e == 0),
                                    stop=(e == EK - 1),
                                )
                            raw = ropep.tile([128, 512], f32r, tag="raw")
                            nc.scalar.copy(raw[:], pp[:])
                            rope_pend.append((ti, dest, sl, pp, raw))
                            if len(rope_pend) > 1:
                                rope_tail()
                while rope_pend:
                    rope_tail()

                # --- v projection, both pairs at once (N=256) ---
                for st in range(N_SK):
                    vp_ps = vpp.tile([128, 2 * 128], f32, tag="vps")
                    for e in range(EK):
                        nc.tensor.matmul(
                            vp_ps[:],
                            xt_sb[e][:, 128 * st : 128 * (st + 1)],
                            wv_sb[e][:],
                            start=(e == 0),
                            stop=(e == EK - 1),
                        )
                    for h in range(4):
                        nc.vector.tensor_copy(
                            v_c[:, st, 65 * h : 65 * h + 64],
                            vp_ps[:, 64 * h : 64 * h + 64],
                        )
                if dbg:
                    nc.sync.dma_start(d_qT0[:], qT[0][:])
                    nc.sync.dma_start(d_kT0[:], kT[0][:])
                    pass

            # ---------------- attention (both pairs) ----------------
            with (
                tc.tile_pool(name="attps", bufs=1, space="PSUM") as attps,
                tc.tile_pool(name="expp", bufs=4) as expp,
                tc.tile_pool(name="recipp", bufs=2) as rcp,
            ):
                pending_norm = [None]
                for g in range(PAIRS):
                    for ch in range(N_CH):
                        cslice = slice(SQ_CHUNK * ch, SQ_CHUNK * (ch + 1))
                        # attn@v accumulators, row-tiled: per head, the K=128
                        # sk contraction splits into two CONCURRENT K=64
                        # matmuls (v rows 0:64 / 64:128 via auto tile_position)
                        # accumulating into separate single-bank PSUM tiles,
                        # summed once at normalization time.
                        oT = {
                            (h, half): attps.tile(
                                [65, SQ_CHUNK], f32, tag=f"oT{h}{half}",
                                name=f"oT{h}{half}",
                            )
                            for h in range(2)
                            for half in range(2)
                        }
                        exps = []  # (eA, eB) per sk, attn@v lags three sk
                        hA, hB = 2 * g, 2 * g + 1

                        def attnv(sk):
                            eA, eB = exps[sk]
                            first = sk == 0
                            last = sk == N_SK - 1
                            for h, e_t in ((0, eA), (1, eB)):
                                hh = 2 * g + h
                                for half in range(2):
                                    psl = slice(64 * half, 64 * half + 64)
                                    nc.tensor.matmul(
                                        oT[(h, half)][:],
                                        v_c[psl, sk, 65 * hh : 65 * hh + 65],
                                        e_t[psl, :],
                                        start=first,
                                        stop=last,
                                    )

                        for sk in range(N_SK):
                            sksl = slice(128 * sk, 128 * (sk + 1))
                            # depth-2 ping-pong on single-bank score slots
                            sA = attps.tile([128, SQ_CHUNK], f32, tag=f"sA{sk % 2}", name="sA")
                            sB = attps.tile([128, SQ_CHUNK], f32, tag=f"sB{sk % 2}", name="sB")
                            # scores: two CONCURRENT row-tiled K=64 matmuls
                            nc.tensor.matmul(
                                sA[:],
                                kT[g][0:64, sksl],
                                qT[g][0:64, cslice],
                                start=True,
                                stop=True,
                            )
                            nc.tensor.matmul(
                                sB[:],
                                kT[g][64:128, sksl],
                                qT[g][64:128, cslice],
                                start=True,
                                stop=True,
                            )
                            # head A exp on ScalarE; head B via Schraudolph
                            # fast-exp (DVE int math + gpsimd bitcast->bf16)
                            eA = expp.tile([128, SQ_CHUNK], bf16, tag="eA", bufs=5)
                            nc.scalar.activation(eA[:], sA[:], AF.Exp, scale=0.125)
                            ei = expp.tile([128, SQ_CHUNK], i32, tag="ei", bufs=3)
                            nc.vector.tensor_scalar(
                                ei[:], sB[:], SCH_A, SCH_B, ALU.mult, ALU.add
                            )
                            eB = expp.tile([128, SQ_CHUNK], bf16, tag="eB", bufs=5)
                            nc.gpsimd.tensor_scalar(
                                eB[:], ei.bitcast(f32), 1.0, 0.0, ALU.mult, ALU.add
                            )
                            exps.append((eA, eB))
                            if sk == 1 and pending_norm[0] is not None:
                                pending_norm[0]()
                                pending_norm[0] = None
                            # PE heater: standalone LDWEIGHTS keeps the PE
                            # activity monitor from re-throttling the clock
                            # during producer waits (every real matmul
                            # self-loads its weights, so this is harmless).
                            nc.tensor.ldweights(v_c[:, 0, 0:128])
                            # attn@v lags 4 sk so the scores->DVE->gpsimd
                            # chain is always complete
                            if sk > 3:
                                attnv(sk - 4)
                        for tsk in range(N_SK - 4, N_SK):
                            attnv(tsk)
                        # Normalization is DEFERRED: emitted at sk==1 of
                        # the NEXT chunk so its evac/add/db/recip chain
                        # latency hides behind independent scores/exp work.
                        def make_norm(g=g, cslice=cslice, oT=oT):
                            def norm():
                                oS = {}
                                for h in range(2):
                                    ohi = rcp.tile([65, SQ_CHUNK], f32r, tag=f"ohi{h}", name="ohi")
                                    nc.scalar.copy(ohi[:], oT[(h, 1)][:])
                                    o_s = rcp.tile([65, SQ_CHUNK], f32r, tag=f"o{h}", name="os")
                                    nc.vector.tensor_add(o_s[:], oT[(h, 0)][:], ohi[:])
                                    oS[h] = o_s
                                for h in range(2):
                                    o_s = oS[h]
                                    db = attps.tile([64, SQ_CHUNK], f32, tag=f"oT{h}0", name="db")
                                    nc.tensor.matmul(
                                        db[:], onesrow[64:65, :], o_s[64:65, :],
                                        start=True, stop=True,
                                    )
                                    rb = rcp.tile([64, SQ_CHUNK], f32, tag=f"rb{h}", name="rb")
                                    nc.vector.reciprocal_approx_fast(rb[:], db[:])
                                    if h == 0:
                                        nc.vector.tensor_mul(
                                            att_o[g][0:64, cslice], o_s[0:64, :], rb[:]
                                        )
                                    else:
                                        aoB = rcp.tile([64, SQ_CHUNK], f32r, tag="aoB", name="aoB")
                                        nc.vector.tensor_mul(aoB[:], o_s[0:64, :], rb[:])
                                        nc.sync.dma_start(att_o[g][64:128, cslice], aoB[:])
                            return norm
                        pending_norm[0] = make_norm()

                if pending_norm[0] is not None:
                    pending_norm[0]()
                    pending_norm[0] = None

                # ---------------- output projection ----------------
                # (emitted inside the attention pools so the op psum tiles
                # rotate through the score slots instead of waiting for the
                # whole attention pool to drain)
                with tc.tile_pool(name="osb", bufs=3) as osbp:
                    for st in range(S // 128):
                        ssl = slice(128 * st, 128 * (st + 1))
                        ot = osbp.tile([128, E], bf16, tag="ot")
                        for n in range(E // 512):
                            nsl = slice(512 * n, 512 * (n + 1))
                            op = attps.tile(
                                [128, 512], f32,
                                tag=("sA0", "sA1", "sB0", "sB1")[(2 * st + n) % 4],
                                name=f"op{st}_{n}",
                            )
                            for g in range(PAIRS):
                                nc.tensor.matmul(
                                    op[:],
                                    att_o[g][:, ssl],
                                    wo_sb[g][:, nsl],
                                    start=(g == 0),
                                    stop=(g == PAIRS - 1),
                                )
                            nc.scalar.copy(ot[:, nsl], op[:])
                        nc.sync.dma_start(out[ssl, :], ot[:])

    nc.compile()
    return nc


def _get_program():
    if "nc" not in _BUILT:
        _BUILT["nc"] = _build_program()
    return _BUILT["nc"]


def _host_inputs(x, W_qkv, W_out):
    """Build the 8 per-core input maps."""
    import ml_dtypes

    f = np.float32
    bf = ml_dtypes.bfloat16
    x = np.asarray(x, dtype=f)
    W_qkv = np.asarray(W_qkv, dtype=f)
    W_out = np.asarray(W_out, dtype=f)

    inv_freq = 1.0 / (ROPE_THETA ** (np.arange(0, D, 2, dtype=np.float64) / D))
    p = np.arange(128)
    freq_row = inv_freq[(p % D) // 2]  # [128]
    ang = freq_row[:, None] * np.arange(S, dtype=np.float64)[None, :]  # [128, S]
    cos_t = np.cos(ang).astype(f)
    sign = np.where(p % 2 == 0, -1.0, 1.0)[:, None]
    sin_t = (np.sin(ang) * sign).astype(f)

    msw = np.zeros((128, 128), dtype=f)
    msw[p, p ^ 1] = 1.0

    maps = []
    for core in range(N_CORES):
        b, hg = divmod(core, HG)
        hs = [HPG * hg + i for i in range(HPG)]
        w_qk = np.concatenate(
            [W_qkv[:, h * D : (h + 1) * D] for h in hs]
            + [W_qkv[:, ATT + h * D : ATT + (h + 1) * D] for h in hs],
            axis=1,
        )
        w_v = np.concatenate(
            [W_qkv[:, 2 * ATT + h * D : 2 * ATT + (h + 1) * D] for h in hs], axis=1
        )
        w_o = np.concatenate([W_out[h * D : (h + 1) * D, :] for h in hs], axis=0)
        maps.append(
            {
                "xT": np.ascontiguousarray(x[b].T).astype(bf),
                "w_qk": np.ascontiguousarray(w_qk).astype(bf),
                "w_v": np.ascontiguousarray(w_v).astype(bf),
                "w_o": np.ascontiguousarray(w_o),
                "cos_t": cos_t,
                "sin_t": sin_t,
                "mswap": msw,
                "ones_in": np.ones((1, 64), dtype=f),
            }
        )
    return maps


def kernel(x, W_qkv, W_out):
    from concourse.bass_utils import run_bass_kernel_spmd

    nc = _get_program()
    maps = _host_inputs(x, W_qkv, W_out)
    res = run_bass_kernel_spmd(nc, maps, core_ids=list(range(N_CORES)))
    out = np.zeros((B, S, E), dtype=np.float32)
    for core in range(N_CORES):
        b = core // HG
        out[b] += np.asarray(res.results[core]["out"], dtype=np.float32)
    return out

